# revision 1
# baseline (speedup 1.0000x reference)
"""Trainium2 Bass kernel for nn_DiscreteCommunication (GNN message passing).

Strategy (8 NeuronCores, SPMD single program, no collectives):
  - Host: sort edges by dst; device d owns dst nodes [2500d, 2500d+2500),
    i.e. 20 windows of 125 consecutive nodes. Edges land on the device that
    owns their dst. Within a device, each window's edges are padded to a
    uniform number of 128-edge blocks (B blocks/window, same on all devices
    so one SPMD program serves all cores).
  - Device phase Z: Z = [feat|h] @ W_enc.T + b_enc over all 20000 nodes
    (replicated on every core) -> Z[20000,128] in local DRAM. featH is
    supplied channel-major from the host so no PE transposes are needed.
    The per-edge encoder is then just a gather of Z rows.
  - Device phase MSG: per window, dma_gather Z[src] for B*128 edges, add
    Gumbel noise g = -log(-log(u+eps)+eps), take pairwise argmax to get the
    binary message m (bf16), build one-hot P[e, slot] = (dst_slot[e]==slot),
    and accumulate c_sumT[j, slot] += m.T @ P on the PE (per 128-edge
    block). c = (c_sum > 0) equals the segment-max of one-hot messages.
    Message columns are permuted evens-first (via W_enc/u on the host,
    undone through W_b row order) so every elementwise op is contiguous
    and window-wide.
  - Device phase GRU: node-parallel GRU over the 2500 owned nodes with
    dec folded in: gi = feat@W_a.T + c@(W_ih[:,128:]@W_dec).T + b_comb.
  - Host: concatenate the 8 per-device h_new slices.
"""
import os
import sys

sys.path.insert(0, "/opt/trn_rl_repo")

import numpy as np
import concourse.bacc as bacc
import concourse.mybir as mybir
import concourse.tile as tile
from concourse.bass_utils import run_bass_kernel_spmd

F32 = mybir.dt.float32
BF16 = mybir.dt.bfloat16
I16 = mybir.dt.int16
AF = mybir.ActivationFunctionType
OP = mybir.AluOpType

N_NODES = 20000
HIDDEN = 128
MSG = 64
TWO_MSG = 2 * MSG  # 128
N_EDGES = 320000
EPS = 1e-10
NDEV = 8
WIN_NODES = 125            # nodes per window (<=128 for one-hot slots)
WINS = 20                  # windows per device
DEV_NODES = WIN_NODES * WINS   # 2500
N_WINDOWS = NDEV * WINS        # 160, covers all 20000 nodes exactly
ZBLKS = (N_NODES + 127) // 128  # 157 blocks over nodes (last partial: 32)
ZPAD = ZBLKS * 128              # 20096
GMAX = 1024                     # dma_gather cap: 64 descriptors/engine

_cache = {}


def build_program(B, phases="zmg", zero_bias=True, repeats=1,
                  skip_gather=False, skip_ln=False, skip_dve=False,
                  skip_zmm=False, skip_zcopy=False, skip_zwrite=False):
    """Build the SPMD Bass program for B blocks-per-window.
    zero_bias: all of b_enc/b_dec/b_ih/b_hh are zero (true for this problem's
    spec); skips the bias adds. The general path is kept for safety."""
    nc = bacc.Bacc("TRN2", target_bir_lowering=False)
    EW = B * 128               # padded edges per window
    EDEV = WINS * EW           # padded edges per device

    # ---- I/O ----
    # channel-major featH: [p, zb, a, n] = featH_pad[zb*128+n, a*128+p]
    fh_hi = nc.dram_tensor("fh_hi", [128, ZBLKS * 256], BF16, kind="ExternalInput")
    fh_lo = nc.dram_tensor("fh_lo", [128, ZBLKS * 256], BF16, kind="ExternalInput")
    fh_locT = nc.dram_tensor("fh_locT", [128, WINS * 256], F32, kind="ExternalInput")
    h_loc = nc.dram_tensor("h_loc", [DEV_NODES, HIDDEN], F32, kind="ExternalInput")
    u_g = nc.dram_tensor("u_g", [128, EDEV], F32, kind="ExternalInput")
    src16 = nc.dram_tensor("src16", [128, EDEV // 16], I16, kind="ExternalInput")
    dstslot = nc.dram_tensor("dstslot", [128, WINS * B], F32, kind="ExternalInput")
    wencT_hi = nc.dram_tensor("wencT_hi", [256, TWO_MSG], BF16, kind="ExternalInput")
    wencT_lo = nc.dram_tensor("wencT_lo", [256, TWO_MSG], BF16, kind="ExternalInput")
    waT = nc.dram_tensor("waT", [128, 384], F32, kind="ExternalInput")
    wbT = nc.dram_tensor("wbT", [128, 384], F32, kind="ExternalInput")
    whhT = nc.dram_tensor("whhT", [128, 384], F32, kind="ExternalInput")
    bias_enc = nc.dram_tensor("bias_enc", [128, 128], F32, kind="ExternalInput")
    bias_rz = nc.dram_tensor("bias_rz", [128, 256], F32, kind="ExternalInput")
    bias_n = nc.dram_tensor("bias_n", [128, 128], F32, kind="ExternalInput")
    bias_hn = nc.dram_tensor("bias_hn", [128, 128], F32, kind="ExternalInput")
    h_new = nc.dram_tensor("h_new", [DEV_NODES, HIDDEN], F32, kind="ExternalOutput")

    with tile.TileContext(nc) as tc:
        with (
            tc.tile_pool(name="const", bufs=1) as cp,
        ):
            # ---- persistent constants ----
            eps_t = cp.tile([128, 1], F32)
            nc.vector.memset(eps_t[:], EPS)
            iota_x = cp.tile([128, EW], F32)
            # values 0..127 repeated B times along free dim; exact in f32
            nc.gpsimd.iota(iota_x[:], pattern=[[0, B], [1, 128]], base=0,
                           channel_multiplier=0,
                           allow_small_or_imprecise_dtypes=True)
            dslot_t = cp.tile([128, WINS * B], F32)
            nc.sync.dma_start(out=dslot_t[:], in_=dstslot[:])
            wh_t = cp.tile([128, 2, TWO_MSG], BF16)
            nc.sync.dma_start(out=wh_t[:], in_=wencT_hi.rearrange("(a p) j -> p a j", p=128))
            wl_t = cp.tile([128, 2, TWO_MSG], BF16)
            nc.sync.dma_start(out=wl_t[:], in_=wencT_lo.rearrange("(a p) j -> p a j", p=128))
            waT_t = cp.tile([128, 384], F32)
            nc.sync.dma_start(out=waT_t[:], in_=waT[:])
            wbT_t = cp.tile([128, 384], F32)
            nc.sync.dma_start(out=wbT_t[:], in_=wbT[:])
            whhT_t = cp.tile([128, 384], F32)
            nc.sync.dma_start(out=whhT_t[:], in_=whhT[:])
            bias_enc_t = cp.tile([128, 128], F32)
            nc.sync.dma_start(out=bias_enc_t[:], in_=bias_enc[:])
            bias_rz_t = cp.tile([128, 256], F32)
            nc.sync.dma_start(out=bias_rz_t[:], in_=bias_rz[:])
            bias_n_t = cp.tile([128, 128], F32)
            nc.sync.dma_start(out=bias_n_t[:], in_=bias_n[:])
            bias_hn_t = cp.tile([128, 128], F32)
            nc.sync.dma_start(out=bias_hn_t[:], in_=bias_hn[:])
            src16_t = cp.tile([128, EDEV // 16], I16)
            nc.sync.dma_start(out=src16_t[:], in_=src16[:])
            cT_tiles = []
            for w in range(WINS):
                ct = cp.tile([128, 128], F32, tag=f"cT{w}")
                cT_tiles.append(ct)

            Z = nc.dram_tensor("Zscr", [ZPAD, TWO_MSG], F32)
            Zv = Z.rearrange("(g p) j -> p g j", p=128)  # [128, ZBLKS, 128]

            # ---- Phase Z: Z = [feat|h] @ W_enc.T + b_enc (all nodes) ----
            # 16 node-blocks per group -> one [128,2048] PSUM (4 banks, slices
            # bank-aligned) + one DVE evacuation op; 2MB featH per group DMA
            ZG = 16
            def emit_z_phase():
             with (
                tc.tile_pool(name="zio", bufs=4) as zio,
                tc.tile_pool(name="zps", bufs=2, space="PSUM") as zps,
             ):
              for g0 in range(0, ZBLKS, ZG):
                gn = min(ZG, ZBLKS - g0)
                fghi = zio.tile([128, ZG, 2, 128], BF16, tag="fghi")
                fglo = zio.tile([128, ZG, 2, 128], BF16, tag="fglo")
                cols = gn * 256
                nc.sync.dma_start(
                    out=fghi[:].rearrange("p g a n -> p (g a n)")[:, :cols],
                    in_=fh_hi[:, g0 * 256 : g0 * 256 + cols])
                nc.scalar.dma_start(
                    out=fglo[:].rearrange("p g a n -> p (g a n)")[:, :cols],
                    in_=fh_lo[:, g0 * 256 : g0 * 256 + cols])
                zp = zps.tile([128, ZG * 128], F32, space="PSUM", tag="zp")
                if not skip_zmm:
                    for zi in range(gn):
                        zslc = zp[:, zi * 128 : (zi + 1) * 128]
                        # hi*hi + hi*lo + lo*hi (lo*lo dropped, ~2^-16 relative)
                        nc.tensor.matmul(out=zslc, lhsT=fghi[:, zi, 0, :],
                                         rhs=wh_t[:, 0, :], start=True, stop=False)
                        nc.tensor.matmul(out=zslc, lhsT=fghi[:, zi, 0, :],
                                         rhs=wl_t[:, 0, :], start=False, stop=False)
                        nc.tensor.matmul(out=zslc, lhsT=fglo[:, zi, 0, :],
                                         rhs=wh_t[:, 0, :], start=False, stop=False)
                        nc.tensor.matmul(out=zslc, lhsT=fghi[:, zi, 1, :],
                                         rhs=wh_t[:, 1, :], start=False, stop=False)
                        nc.tensor.matmul(out=zslc, lhsT=fghi[:, zi, 1, :],
                                         rhs=wl_t[:, 1, :], start=False, stop=False)
                        nc.tensor.matmul(out=zslc, lhsT=fglo[:, zi, 1, :],
                                         rhs=wh_t[:, 1, :], start=False, stop=True)
                else:
                    nc.vector.memset(zp[:, : gn * 128], 0.0)
                zs = zio.tile([128, ZG, 128], F32, tag="zs")
                if skip_zcopy:
                    pass
                elif zero_bias:
                    nc.scalar.copy(out=zs[:, :gn, :].rearrange("p g j -> p (g j)"),
                                   in_=zp[:, : gn * 128])
                else:
                    nc.vector.tensor_tensor(
                        out=zs[:, :gn, :],
                        in0=zp[:, : gn * 128].rearrange("p (g j) -> p g j", g=gn),
                        in1=bias_enc_t[:, None, :].to_broadcast([128, gn, 128]),
                        op=OP.add)
                if not skip_zwrite:
                    # SWDGE path: keeps both HWDGE rings free for featH reads
                    nc.gpsimd.dma_start(out=Zv[:, g0 : g0 + gn, :], in_=zs[:, :gn, :])

            # ---- Phase MSG + GRU, interleaved ----
            WG = 4
            def emit_msg_window(w):
                zg = gp.tile([128, B, TWO_MSG], F32, tag="zg")
                if not skip_gather:
                    for off in range(0, EW, GMAX):
                        chunk = min(GMAX, EW - off)
                        nc.gpsimd.dma_gather(
                            zg[:, off // 128 : (off + chunk) // 128, :], Z[:],
                            src16_t[:, (w * EW + off) // 16 : (w * EW + off + chunk) // 16],
                            num_idxs=chunk, num_idxs_reg=chunk, elem_size=TWO_MSG,
                        )
                uw = gp.tile([128, EW], F32, tag="uw")
                ueng = nc.sync if w % 2 == 0 else nc.scalar
                ueng.dma_start(out=uw[:], in_=u_g[:, w * EW : (w + 1) * EW])
                # gumbel: t1 = ln(u+eps); t2 = ln(-t1+eps); a = z - t2
                t1 = mp.tile([128, EW], F32, tag="t1")
                t2 = mp.tile([128, EW], F32, tag="t2")
                if not skip_ln:
                    nc.scalar.activation(t1[:], uw[:], AF.Ln, bias=eps_t[:, :1], scale=1.0)
                    nc.scalar.activation(t2[:], t1[:], AF.Ln, bias=eps_t[:, :1], scale=-1.0)
                a = mp.tile([128, B, 128], F32, tag="a")
                if not skip_dve:
                    nc.vector.tensor_tensor(out=a[:].rearrange("p b j -> p (b j)"),
                                            in0=zg[:].rearrange("p b j -> p (b j)"),
                                            in1=t2[:], op=OP.subtract)
                # columns are evens-first: option0 = [:, :, :64], option1 = [:, :, 64:]
                m = mp.tile([128, B, 128], BF16, tag="m")
                nc.vector.tensor_tensor(out=m[:, :, 0:64], in0=a[:, :, 0:64],
                                        in1=a[:, :, 64:128], op=OP.is_ge)
                # m1 = 1 - m0 == (m0 < 1)
                nc.vector.tensor_scalar(out=m[:, :, 64:128], in0=m[:, :, 0:64],
                                        scalar1=1.0, scalar2=None, op0=OP.is_lt)
                P = pp.tile([128, B, 128], BF16, tag="P")
                nc.vector.tensor_tensor(
                    out=P[:],
                    in0=iota_x[:].rearrange("p (b j) -> p b j", b=B),
                    in1=dslot_t[:, w * B : (w + 1) * B, None].to_broadcast([128, B, 128]),
                    op=OP.is_equal)
                cps = mps.tile([128, 128], F32, space="PSUM", tag="cps")
                for b in range(B):
                    nc.tensor.matmul(out=cps[:], lhsT=m[:, b, :], rhs=P[:, b, :],
                                     start=(b == 0), stop=(b == B - 1))
                # c = (c_sum > 0) == Sign(c_sum) since c_sum >= 0; runs on ACT
                nc.scalar.sign(out=cT_tiles[w][:], in_=cps[:])

            def emit_gru_group(w0):
                xh = rp.tile([128, WG, 2, 128], F32, tag="xh")
                nc.sync.dma_start(
                    out=xh[:].rearrange("p w a n -> p (w a n)"),
                    in_=fh_locT[:, w0 * 256 : (w0 + WG) * 256])
                hl = rp.tile([128, WG, 128], F32, tag="hl")
                for wi in range(WG):
                    w = w0 + wi
                    nc.sync.dma_start(
                        out=hl[:WIN_NODES, wi, :],
                        in_=h_loc[w * WIN_NODES : (w + 1) * WIN_NODES, :])
                # pad each window's slice to 512 f32 = one full PSUM bank so no
                # matmul output crosses a bank boundary
                gi = rps.tile([128, WG, 512], F32, space="PSUM", tag="gi")
                hn_ps = rps2.tile([128, WG, 128], F32, space="PSUM", tag="hn_ps")
                for wi in range(WG):
                    w = w0 + wi
                    nc.tensor.matmul(out=gi[:, wi, 0:384], lhsT=xh[:, wi, 0, :],
                                     rhs=waT_t[:], start=True, stop=False)
                    nc.tensor.matmul(out=gi[:, wi, 0:384], lhsT=cT_tiles[w][:],
                                     rhs=wbT_t[:], start=False, stop=False)
                    nc.tensor.matmul(out=gi[:, wi, 0:256], lhsT=xh[:, wi, 1, :],
                                     rhs=whhT_t[:, 0:256], start=False, stop=True,
                                     skip_group_check=True)
                    nc.tensor.matmul(out=hn_ps[:, wi, :], lhsT=xh[:, wi, 1, :],
                                     rhs=whhT_t[:, 256:384], start=True, stop=True)
                rz_s = rp.tile([128, WG, 256], F32, tag="rz_s")
                if zero_bias:
                    nc.scalar.activation(rz_s[:], gi[:, :, 0:256], AF.Sigmoid)
                    rhn = rp.tile([128, WG, 128], F32, tag="rhn")
                    nc.vector.tensor_tensor(out=rhn[:], in0=rz_s[:, :, 0:128],
                                            in1=hn_ps[:], op=OP.mult)
                    narg = rp.tile([128, WG, 128], F32, tag="narg")
                    nc.vector.tensor_tensor(out=narg[:], in0=rhn[:],
                                            in1=gi[:, :, 256:384], op=OP.add)
                else:
                    rz = rp.tile([128, WG, 256], F32, tag="rz")
                    nc.vector.tensor_tensor(
                        out=rz[:], in0=gi[:, :, 0:256],
                        in1=bias_rz_t[:, None, :].to_broadcast([128, WG, 256]), op=OP.add)
                    nc.scalar.activation(rz_s[:], rz[:], AF.Sigmoid)
                    hn = rp.tile([128, WG, 128], F32, tag="hn")
                    nc.vector.tensor_tensor(
                        out=hn[:], in0=hn_ps[:],
                        in1=bias_hn_t[:, None, :].to_broadcast([128, WG, 128]), op=OP.add)
                    inn = rp.tile([128, WG, 128], F32, tag="inn")
                    nc.vector.tensor_tensor(
                        out=inn[:], in0=gi[:, :, 256:384],
                        in1=bias_n_t[:, None, :].to_broadcast([128, WG, 128]), op=OP.add)
                    rhn = rp.tile([128, WG, 128], F32, tag="rhn")
                    nc.vector.tensor_tensor(out=rhn[:], in0=rz_s[:, :, 0:128], in1=hn[:], op=OP.mult)
                    narg = rp.tile([128, WG, 128], F32, tag="narg")
                    nc.vector.tensor_tensor(out=narg[:], in0=inn[:], in1=rhn[:], op=OP.add)
                n_t = rp.tile([128, WG, 128], F32, tag="n_t")
                nc.scalar.activation(n_t[:], narg[:], AF.Tanh)
                hmn = rp.tile([128, WG, 128], F32, tag="hmn")
                nc.vector.tensor_tensor(out=hmn[:], in0=hl[:], in1=n_t[:], op=OP.subtract)
                zh = rp.tile([128, WG, 128], F32, tag="zh")
                nc.vector.tensor_tensor(out=zh[:], in0=rz_s[:, :, 128:256], in1=hmn[:], op=OP.mult)
                ho = rp.tile([128, WG, 128], F32, tag="ho")
                nc.vector.tensor_tensor(out=ho[:], in0=n_t[:], in1=zh[:], op=OP.add)
                for wi in range(WG):
                    w = w0 + wi
                    nc.sync.dma_start(
                        out=h_new[w * WIN_NODES : (w + 1) * WIN_NODES, :],
                        in_=ho[:WIN_NODES, wi, :])

            for _rep in range(repeats):
                if "z" in phases:
                    emit_z_phase()
                with (
                    tc.tile_pool(name="msg", bufs=2) as mp,
                    tc.tile_pool(name="ponehot", bufs=3) as pp,
                    tc.tile_pool(name="gat", bufs=3) as gp,
                    tc.tile_pool(name="mps", bufs=2, space="PSUM") as mps,
                    tc.tile_pool(name="gru", bufs=2) as rp,
                    tc.tile_pool(name="rps", bufs=1, space="PSUM") as rps,
                    tc.tile_pool(name="rps2", bufs=2, space="PSUM") as rps2,
                ):
                    for w in range(WINS):
                        if "m" in phases:
                            emit_msg_window(w)
                        if "g" in phases and w % WG == WG - 1:
                            emit_gru_group(w - WG + 1)

    nc.compile()
    return nc


# message-column permutation: evens first, then odds
PERM = np.concatenate([np.arange(0, TWO_MSG, 2), np.arange(1, TWO_MSG, 2)])


def _prep_host(feat, h, src, dst, u):
    """Host-side sharding/layout. Returns (B, list of per-core in_maps)."""
    feat = np.ascontiguousarray(feat, dtype=np.float32)
    h = np.ascontiguousarray(h, dtype=np.float32)
    src = np.asarray(src).astype(np.int64)
    dst = np.asarray(dst).astype(np.int64)
    u2 = np.ascontiguousarray(u, dtype=np.float32).reshape(N_EDGES, TWO_MSG)
    u2 = u2[:, PERM]

    import ml_dtypes
    bf16 = ml_dtypes.bfloat16
    featH = np.concatenate([feat, h], axis=1)  # [N, 256]
    featH_pad = np.zeros((ZPAD, 256), np.float32)
    featH_pad[:N_NODES] = featH

    def _swz(x):
        # channel-major: [p, zb, a, n] = x[zb*128+n, a*128+p]
        return np.ascontiguousarray(
            x.reshape(ZBLKS, 128, 2, 128).transpose(3, 0, 2, 1)).reshape(128, -1)

    fhi = featH_pad.astype(bf16)
    flo = (featH_pad - fhi.astype(np.float32)).astype(bf16)
    fh_hi = _swz(fhi)
    fh_lo = _swz(flo)

    order = np.argsort(dst, kind="stable")
    dst_s = dst[order]
    src_s = src[order]
    win = dst_s // WIN_NODES                     # window id per sorted edge
    counts = np.bincount(win, minlength=N_WINDOWS)
    starts = np.zeros(N_WINDOWS + 1, np.int64)
    np.cumsum(counts, out=starts[1:])
    B = int(np.max((counts + 127) // 128))
    B = max(B, 1)
    EW = B * 128
    EDEV = WINS * EW

    in_maps = []
    for d in range(NDEV):
        src_pad = np.zeros((EDEV,), np.int64)
        slot_pad = np.full((EDEV,), -1.0, np.float32)
        u_pad = np.full((EDEV, TWO_MSG), 0.5, np.float32)
        for k in range(WINS):
            wid = d * WINS + k
            s, e = starts[wid], starts[wid + 1]
            n = e - s
            o = k * EW
            src_pad[o : o + n] = src_s[s:e]
            slot_pad[o : o + n] = (dst_s[s:e] - wid * WIN_NODES).astype(np.float32)
            u_pad[o : o + n] = u2[order[s:e]]

        # gather idx layout: [p, s] = idx[16*s + p%16], replicated across groups
        idx16 = np.empty((128, EDEV // 16), np.int16)
        flat = src_pad.astype(np.int16).reshape(EDEV // 16, 16).T  # [16, EDEV/16]
        for g in range(8):
            idx16[g * 16 : (g + 1) * 16, :] = flat
        # compact dstslot: [p, w*B + b] = slot of edge (w, b, p)
        dstslot_c = np.ascontiguousarray(slot_pad.reshape(WINS * B, 128).T)
        # u swizzled: [p, blk*128 + c] = u_pad[blk*128 + p, c]
        u_sw = np.ascontiguousarray(
            u_pad.reshape(EDEV // 128, 128, TWO_MSG).transpose(1, 0, 2).reshape(128, -1))
        # local featH channel-major: [p, w, a, n] = featH[2500d+125w+n, a*128+p]
        base = d * DEV_NODES
        loc = np.zeros((WINS, 128, 2, 128), np.float32)  # [w, n, a, p]
        loc[:, :WIN_NODES] = featH[base : base + DEV_NODES].reshape(
            WINS, WIN_NODES, 2, 128)
        fh_locT = np.ascontiguousarray(loc.transpose(3, 0, 2, 1)).reshape(128, -1)
        h_loc = np.ascontiguousarray(h[base : base + DEV_NODES])
        in_maps.append({
            "fh_hi": fh_hi, "fh_lo": fh_lo, "fh_locT": fh_locT, "h_loc": h_loc,
            "u_g": u_sw, "src16": idx16, "dstslot": dstslot_c,
        })
    return B, in_maps


def _prep_weights(W_enc, b_enc, W_dec, b_dec, W_ih, W_hh, b_ih, b_hh):
    W_enc = np.asarray(W_enc, np.float32)
    W_dec = np.asarray(W_dec, np.float32)
    W_ih = np.asarray(W_ih, np.float32)
    W_hh = np.asarray(W_hh, np.float32)
    b_enc = np.asarray(b_enc, np.float32)
    b_dec = np.asarray(b_dec, np.float32)
    b_ih = np.asarray(b_ih, np.float32)
    b_hh = np.asarray(b_hh, np.float32)

    W_b = (W_ih[:, HIDDEN:].astype(np.float64) @ W_dec.astype(np.float64))
    b_comb = (W_ih[:, HIDDEN:].astype(np.float64) @ b_dec.astype(np.float64)) + b_ih

    import ml_dtypes
    bf16 = ml_dtypes.bfloat16
    wencT = np.ascontiguousarray(W_enc.T[:, PERM])           # [256, 128] permuted cols
    wencT_hi = wencT.astype(bf16)
    wencT_lo = (wencT - wencT_hi.astype(np.float32)).astype(bf16)
    waT = np.ascontiguousarray(W_ih[:, :HIDDEN].T)           # [128, 384]
    wbT = np.ascontiguousarray(W_b.T.astype(np.float32)[PERM, :])  # [128, 384] perm rows
    whhT = np.ascontiguousarray(W_hh.T)                      # [128, 384]
    brz = (b_comb[:256] + b_hh[:256]).astype(np.float32)
    bn = b_comb[256:384].astype(np.float32)
    bhn = b_hh[256:384].astype(np.float32)
    return {
        "wencT_hi": wencT_hi, "wencT_lo": wencT_lo,
        "waT": waT, "wbT": wbT, "whhT": whhT,
        "bias_enc": np.ascontiguousarray(np.tile(b_enc[PERM], (128, 1))),
        "bias_rz": np.ascontiguousarray(np.tile(brz, (128, 1))),
        "bias_n": np.ascontiguousarray(np.tile(bn, (128, 1))),
        "bias_hn": np.ascontiguousarray(np.tile(bhn, (128, 1))),
    }


def kernel(feat, h, src, dst, u, W_enc, b_enc, W_dec, b_dec, W_ih, W_hh,
           b_ih, b_hh):
    B, in_maps = _prep_host(feat, h, src, dst, u)
    wmap = _prep_weights(W_enc, b_enc, W_dec, b_dec, W_ih, W_hh, b_ih, b_hh)
    for m in in_maps:
        m.update(wmap)

    phases = os.environ.get("KERNEL_PHASES", "zmg")
    zero_bias = not (np.any(np.asarray(b_enc)) or np.any(np.asarray(b_dec))
                     or np.any(np.asarray(b_ih)) or np.any(np.asarray(b_hh)))
    key = (B, phases, zero_bias)
    if key not in _cache:
        _cache[key] = build_program(B, phases, zero_bias)
    nc = _cache[key]

    res = run_bass_kernel_spmd(nc, in_maps, core_ids=list(range(NDEV)))
    h_new = np.concatenate([res.results[d]["h_new"] for d in range(NDEV)], axis=0)
    return (h_new, h_new)



# revision 3
# speedup vs baseline: 2.3478x; 2.3478x over previous
"""Trainium2 Bass kernel for nn_DiscreteCommunication (GNN message passing).

Strategy (8 NeuronCores, SPMD single program, no collectives):
  - Host: sort edges by dst; device d owns dst nodes [2500d, 2500d+2500),
    i.e. 20 windows of 125 consecutive nodes. Edges land on the device that
    owns their dst; within each window edges are sorted by src (sequential-ish
    gather addresses) and padded to B 128-edge blocks (B global so one SPMD
    program serves all cores).
  - Gumbel reformulation: m0 = argmax0 <=> d >= t2_e - t2_o where
    t2 = ln(-ln(u+eps)+eps), d = z_e - z_o. Equivalently, with
    E = exp(d) and t1 = ln(u+eps) (<= 0):  m0 = (E * t1_o <= t1_e).
    This removes the second Ln pass entirely; only one Ln per u element.
  - Device phase E: D = [feat|h] @ W_diff.T over all 20000 nodes (replicated,
    bf16 single-term matmul), E = exp(D) applied during PSUM evacuation,
    stored as bf16 [ZPAD, 128] rows (64 data + 64 zero pad -> 256B rows for
    the gather's descriptor-size constraint).
  - Device phase MSG: per window, one dma_gather of E[src] (256B rows,
    src-sorted), t1 = Ln(u+eps) from bf16 u, prod/is_le at 2x bf16 DVE rate,
    one-hot P (is_equal, split DVE/Pool), c_sumT += m.T @ P on PE per
    128-edge block, c = sign(c_sum) on ACT.
  - Device phase GRU: node-parallel bf16 GRU over the 2500 owned nodes with
    the decoder folded in: gi = feat@W_a.T + c@(W_ih[:,128:]@W_dec).T.
  - Host: concatenate the 8 per-device h_new slices.
"""
import os
import sys

sys.path.insert(0, "/opt/trn_rl_repo")

import numpy as np
import concourse.bacc as bacc
import concourse.mybir as mybir
import concourse.tile as tile
from concourse.bass_utils import run_bass_kernel_spmd

F32 = mybir.dt.float32
BF16 = mybir.dt.bfloat16
I16 = mybir.dt.int16
AF = mybir.ActivationFunctionType
OP = mybir.AluOpType

N_NODES = 20000
HIDDEN = 128
MSG = 64
TWO_MSG = 2 * MSG  # 128
N_EDGES = 320000
EPS = 1e-10
NDEV = 8
WIN_NODES = 125            # nodes per window (<=128 for one-hot slots)
WINS = 20                  # windows per device
DEV_NODES = WIN_NODES * WINS   # 2500
N_WINDOWS = NDEV * WINS        # 160, covers all 20000 nodes exactly
ZBLKS = (N_NODES + 127) // 128  # 157 blocks over nodes (last partial: 32)
ZPAD = ZBLKS * 128              # 20096

_cache = {}


def build_program(B, phases="zmg", zero_bias=True, repeats=1,
                  skip_gather=False, skip_ln=False, skip_dve=False,
                  skip_zmm=False, skip_zcopy=False, skip_zwrite=False,
                  skip_udma=False, pool_p_mod=1):
    """Build the SPMD Bass program for B blocks-per-window.
    pool_p_mod: windows with w % pool_p_mod == 1 build the one-hot P on the
    Pool engine (gpsimd) instead of DVE, to balance engine load."""
    nc = bacc.Bacc("TRN2", target_bir_lowering=False)
    EW = B * 128               # padded edges per window
    EDEV = WINS * EW           # padded edges per device

    # ---- I/O ----
    # channel-major featH: [p, zb, a, n] = featH_pad[zb*128+n, a*128+p]  (bf16)
    fh = nc.dram_tensor("fh", [128, ZBLKS * 256], BF16, kind="ExternalInput")
    fh_locT = nc.dram_tensor("fh_locT", [128, WINS * 256], BF16, kind="ExternalInput")
    h_loc = nc.dram_tensor("h_loc", [DEV_NODES, HIDDEN], F32, kind="ExternalInput")
    u_g = nc.dram_tensor("u_g", [128, EDEV], BF16, kind="ExternalInput")
    src16 = nc.dram_tensor("src16", [128, EDEV // 16], I16, kind="ExternalInput")
    dstslot = nc.dram_tensor("dstslot", [128, WINS * B], F32, kind="ExternalInput")
    wdT = nc.dram_tensor("wdT", [256, MSG], BF16, kind="ExternalInput")
    waT = nc.dram_tensor("waT", [128, 384], BF16, kind="ExternalInput")
    wbT = nc.dram_tensor("wbT", [128, 384], BF16, kind="ExternalInput")
    whhT = nc.dram_tensor("whhT", [128, 384], BF16, kind="ExternalInput")
    bias_enc = nc.dram_tensor("bias_enc", [128, MSG], F32, kind="ExternalInput")
    bias_rz = nc.dram_tensor("bias_rz", [128, 256], F32, kind="ExternalInput")
    bias_n = nc.dram_tensor("bias_n", [128, 128], F32, kind="ExternalInput")
    bias_hn = nc.dram_tensor("bias_hn", [128, 128], F32, kind="ExternalInput")
    h_new = nc.dram_tensor("h_new", [DEV_NODES, HIDDEN], F32, kind="ExternalOutput")

    with tile.TileContext(nc) as tc:
        with (
            tc.tile_pool(name="const", bufs=1) as cp,
        ):
            # ---- persistent constants ----
            eps_t = cp.tile([128, 1], F32)
            nc.vector.memset(eps_t[:], EPS)
            iota_x = cp.tile([128, EW], F32)
            # values 0..127 repeated B times along free dim; exact in f32
            nc.gpsimd.iota(iota_x[:], pattern=[[0, B], [1, 128]], base=0,
                           channel_multiplier=0,
                           allow_small_or_imprecise_dtypes=True)
            dslot_t = cp.tile([128, WINS * B], F32)
            nc.sync.dma_start(out=dslot_t[:], in_=dstslot[:])
            wdT_t = cp.tile([128, 2, MSG], BF16)
            nc.sync.dma_start(out=wdT_t[:], in_=wdT.rearrange("(a p) j -> p a j", p=128))
            waT_t = cp.tile([128, 384], BF16)
            nc.sync.dma_start(out=waT_t[:], in_=waT[:])
            wbT_t = cp.tile([128, 384], BF16)
            nc.sync.dma_start(out=wbT_t[:], in_=wbT[:])
            whhT_t = cp.tile([128, 384], BF16)
            nc.sync.dma_start(out=whhT_t[:], in_=whhT[:])
            bias_enc_t = cp.tile([128, MSG], F32)
            nc.sync.dma_start(out=bias_enc_t[:], in_=bias_enc[:])
            bias_rz_t = cp.tile([128, 256], F32)
            nc.sync.dma_start(out=bias_rz_t[:], in_=bias_rz[:])
            bias_n_t = cp.tile([128, 128], F32)
            nc.sync.dma_start(out=bias_n_t[:], in_=bias_n[:])
            bias_hn_t = cp.tile([128, 128], F32)
            nc.sync.dma_start(out=bias_hn_t[:], in_=bias_hn[:])
            src16_t = cp.tile([128, EDEV // 16], I16)
            nc.sync.dma_start(out=src16_t[:], in_=src16[:])
            cT_tiles = []
            for w in range(WINS):
                ct = cp.tile([128, 128], BF16, tag=f"cT{w}")
                cT_tiles.append(ct)

            # E table: bf16 rows of 256B = [E (64 cols) | zero pad (64 cols)]
            E = nc.dram_tensor("Escr", [ZPAD, TWO_MSG], BF16)
            Ev = E.rearrange("(g p) j -> p g j", p=128)  # [128, ZBLKS, 128]

            # ---- Phase E: D = [feat|h] @ W_diff.T; E = exp(D) (all nodes) ----
            ZG = 16
            def emit_e_phase():
             with (
                tc.tile_pool(name="zio", bufs=4) as zio,
                tc.tile_pool(name="zps", bufs=2, space="PSUM") as zps,
             ):
              for g0 in range(0, ZBLKS, ZG):
                gn = min(ZG, ZBLKS - g0)
                fg = zio.tile([128, ZG, 2, 128], BF16, tag="fg")
                cols = gn * 256
                half = (cols // 2) // 128 * 128
                nc.sync.dma_start(
                    out=fg[:].rearrange("p g a n -> p (g a n)")[:, :half],
                    in_=fh[:, g0 * 256 : g0 * 256 + half])
                nc.scalar.dma_start(
                    out=fg[:].rearrange("p g a n -> p (g a n)")[:, half:cols],
                    in_=fh[:, g0 * 256 + half : g0 * 256 + cols])
                zp = zps.tile([128, ZG * MSG], F32, space="PSUM", tag="zp")
                if not skip_zmm:
                    for zi in range(gn):
                        zslc = zp[:, zi * MSG : (zi + 1) * MSG]
                        nc.tensor.matmul(out=zslc, lhsT=fg[:, zi, 0, :],
                                         rhs=wdT_t[:, 0, :], start=True, stop=False)
                        nc.tensor.matmul(out=zslc, lhsT=fg[:, zi, 1, :],
                                         rhs=wdT_t[:, 1, :], start=False, stop=True)
                else:
                    nc.vector.memset(zp[:, : gn * MSG], 0.0)
                zs = zio.tile([128, ZG, 128], BF16, tag="zs")
                # zero the pad half (bf16 2x); data half written by Exp below
                nc.vector.memset(zs[:, :gn, MSG:], 0.0)
                if skip_zcopy:
                    pass
                elif zero_bias:
                    nc.scalar.activation(
                        zs[:, :gn, :MSG],
                        zp[:, : gn * MSG].rearrange("p (g j) -> p g j", g=gn),
                        AF.Exp)
                else:
                    zb = zio.tile([128, ZG, MSG], F32, tag="zb")
                    nc.vector.tensor_tensor(
                        out=zb[:, :gn, :],
                        in0=zp[:, : gn * MSG].rearrange("p (g j) -> p g j", g=gn),
                        in1=bias_enc_t[:, None, :].to_broadcast([128, gn, MSG]),
                        op=OP.add)
                    nc.scalar.activation(zs[:, :gn, :MSG], zb[:, :gn, :], AF.Exp)
                if not skip_zwrite:
                    # SWDGE path: keeps both HWDGE rings free for featH reads
                    nc.gpsimd.dma_start(out=Ev[:, g0 : g0 + gn, :], in_=zs[:, :gn, :])

            # ---- Phase MSG + GRU, interleaved ----
            WG = 4
            def emit_msg_window(w):
                zg = gp.tile([128, B, TWO_MSG], BF16, tag="zg")
                if not skip_gather:
                    nc.gpsimd.dma_gather(
                        zg[:], E[:],
                        src16_t[:, (w * EW) // 16 : ((w + 1) * EW) // 16],
                        num_idxs=EW, num_idxs_reg=EW, elem_size=TWO_MSG,
                        single_packet=False,
                    )
                uw = gp.tile([128, EW], BF16, tag="uw")
                if not skip_udma:
                    ueng = nc.sync if w % 2 == 0 else nc.scalar
                    ueng.dma_start(out=uw[:], in_=u_g[:, w * EW : (w + 1) * EW])
                t1 = mp.tile([128, B, 128], BF16, tag="t1")
                if not skip_ln:
                    nc.scalar.activation(t1[:].rearrange("p b c -> p (b c)"), uw[:],
                                         AF.Ln, bias=eps_t[:, :1], scale=1.0)
                m = mp.tile([128, B, 128], BF16, tag="m")
                if not skip_dve:
                    prod = mp.tile([128, B, MSG], BF16, tag="prod")
                    nc.vector.tensor_tensor(out=prod[:], in0=zg[:, :, :MSG],
                                            in1=t1[:, :, MSG:], op=OP.mult)
                    nc.vector.tensor_tensor(out=m[:, :, :MSG], in0=prod[:],
                                            in1=t1[:, :, :MSG], op=OP.is_le)
                    nc.vector.tensor_scalar(out=m[:, :, MSG:], in0=m[:, :, :MSG],
                                            scalar1=1.0, scalar2=None, op0=OP.is_lt)
                P = pp.tile([128, B, 128], BF16, tag="P")
                peng = nc.gpsimd if (w % pool_p_mod == 1) else nc.vector
                peng.tensor_tensor(
                    out=P[:],
                    in0=iota_x[:].rearrange("p (b j) -> p b j", b=B),
                    in1=dslot_t[:, w * B : (w + 1) * B, None].to_broadcast([128, B, 128]),
                    op=OP.is_equal)
                cps = mps.tile([128, 128], F32, space="PSUM", tag="cps")
                for b in range(B):
                    nc.tensor.matmul(out=cps[:], lhsT=m[:, b, :], rhs=P[:, b, :],
                                     start=(b == 0), stop=(b == B - 1))
                # c = (c_sum > 0) == Sign(c_sum) since c_sum >= 0; runs on ACT
                nc.scalar.sign(out=cT_tiles[w][:], in_=cps[:])

            def emit_gru_group(w0):
                xh = rp.tile([128, WG, 2, 128], BF16, tag="xh")
                nc.sync.dma_start(
                    out=xh[:].rearrange("p w a n -> p (w a n)"),
                    in_=fh_locT[:, w0 * 256 : (w0 + WG) * 256])
                hl = rp.tile([128, WG, 128], F32, tag="hl")
                for wi in range(WG):
                    w = w0 + wi
                    nc.sync.dma_start(
                        out=hl[:WIN_NODES, wi, :],
                        in_=h_loc[w * WIN_NODES : (w + 1) * WIN_NODES, :])
                # pad each window's slice to 512 f32 = one full PSUM bank so no
                # matmul output crosses a bank boundary
                gi = rps.tile([128, WG, 512], F32, space="PSUM", tag="gi")
                hn_ps = rps2.tile([128, WG, 128], F32, space="PSUM", tag="hn_ps")
                for wi in range(WG):
                    w = w0 + wi
                    nc.tensor.matmul(out=gi[:, wi, 0:384], lhsT=xh[:, wi, 0, :],
                                     rhs=waT_t[:], start=True, stop=False)
                    nc.tensor.matmul(out=gi[:, wi, 0:384], lhsT=cT_tiles[w][:],
                                     rhs=wbT_t[:], start=False, stop=False)
                    nc.tensor.matmul(out=gi[:, wi, 0:256], lhsT=xh[:, wi, 1, :],
                                     rhs=whhT_t[:, 0:256], start=False, stop=True,
                                     skip_group_check=True)
                    nc.tensor.matmul(out=hn_ps[:, wi, :], lhsT=xh[:, wi, 1, :],
                                     rhs=whhT_t[:, 256:384], start=True, stop=True)
                rz_s = rp.tile([128, WG, 256], F32, tag="rz_s")
                if zero_bias:
                    nc.scalar.activation(rz_s[:], gi[:, :, 0:256], AF.Sigmoid)
                    rhn = rp.tile([128, WG, 128], F32, tag="rhn")
                    nc.vector.tensor_tensor(out=rhn[:], in0=rz_s[:, :, 0:128],
                                            in1=hn_ps[:], op=OP.mult)
                    narg = rp.tile([128, WG, 128], F32, tag="narg")
                    nc.vector.tensor_tensor(out=narg[:], in0=rhn[:],
                                            in1=gi[:, :, 256:384], op=OP.add)
                else:
                    rz = rp.tile([128, WG, 256], F32, tag="rz")
                    nc.vector.tensor_tensor(
                        out=rz[:], in0=gi[:, :, 0:256],
                        in1=bias_rz_t[:, None, :].to_broadcast([128, WG, 256]), op=OP.add)
                    nc.scalar.activation(rz_s[:], rz[:], AF.Sigmoid)
                    hn = rp.tile([128, WG, 128], F32, tag="hn")
                    nc.vector.tensor_tensor(
                        out=hn[:], in0=hn_ps[:],
                        in1=bias_hn_t[:, None, :].to_broadcast([128, WG, 128]), op=OP.add)
                    inn = rp.tile([128, WG, 128], F32, tag="inn")
                    nc.vector.tensor_tensor(
                        out=inn[:], in0=gi[:, :, 256:384],
                        in1=bias_n_t[:, None, :].to_broadcast([128, WG, 128]), op=OP.add)
                    rhn = rp.tile([128, WG, 128], F32, tag="rhn")
                    nc.vector.tensor_tensor(out=rhn[:], in0=rz_s[:, :, 0:128], in1=hn[:], op=OP.mult)
                    narg = rp.tile([128, WG, 128], F32, tag="narg")
                    nc.vector.tensor_tensor(out=narg[:], in0=inn[:], in1=rhn[:], op=OP.add)
                n_t = rp.tile([128, WG, 128], F32, tag="n_t")
                nc.scalar.activation(n_t[:], narg[:], AF.Tanh)
                hmn = rp.tile([128, WG, 128], F32, tag="hmn")
                nc.vector.tensor_tensor(out=hmn[:], in0=hl[:], in1=n_t[:], op=OP.subtract)
                zh = rp.tile([128, WG, 128], F32, tag="zh")
                nc.vector.tensor_tensor(out=zh[:], in0=rz_s[:, :, 128:256], in1=hmn[:], op=OP.mult)
                ho = rp.tile([128, WG, 128], F32, tag="ho")
                nc.vector.tensor_tensor(out=ho[:], in0=n_t[:], in1=zh[:], op=OP.add)
                for wi in range(WG):
                    w = w0 + wi
                    nc.sync.dma_start(
                        out=h_new[w * WIN_NODES : (w + 1) * WIN_NODES, :],
                        in_=ho[:WIN_NODES, wi, :])

            for _rep in range(repeats):
                if "z" in phases:
                    emit_e_phase()
                with (
                    tc.tile_pool(name="msg", bufs=3) as mp,
                    tc.tile_pool(name="ponehot", bufs=3) as pp,
                    tc.tile_pool(name="gat", bufs=3) as gp,
                    tc.tile_pool(name="mps", bufs=2, space="PSUM") as mps,
                    tc.tile_pool(name="gru", bufs=2) as rp,
                    tc.tile_pool(name="rps", bufs=1, space="PSUM") as rps,
                    tc.tile_pool(name="rps2", bufs=2, space="PSUM") as rps2,
                ):
                    for w in range(WINS):
                        if "m" in phases:
                            emit_msg_window(w)
                        if "g" in phases and w % WG == WG - 1:
                            emit_gru_group(w - WG + 1)

    nc.compile()
    return nc


# message-column permutation: evens first, then odds
PERM = np.concatenate([np.arange(0, TWO_MSG, 2), np.arange(1, TWO_MSG, 2)])


def _prep_host(feat, h, src, dst, u):
    """Host-side sharding/layout. Returns (B, list of per-core in_maps)."""
    import ml_dtypes
    bf16 = ml_dtypes.bfloat16

    feat = np.ascontiguousarray(feat, dtype=np.float32)
    h = np.ascontiguousarray(h, dtype=np.float32)
    src = np.asarray(src).astype(np.int64)
    dst = np.asarray(dst).astype(np.int64)
    u2 = np.ascontiguousarray(u, dtype=np.float32).reshape(N_EDGES, TWO_MSG)
    u2 = u2[:, PERM].astype(bf16)

    featH = np.concatenate([feat, h], axis=1)  # [N, 256]
    featH_pad = np.zeros((ZPAD, 256), np.float32)
    featH_pad[:N_NODES] = featH

    # channel-major: [p, zb, a, n] = x[zb*128+n, a*128+p]
    fh = np.ascontiguousarray(
        featH_pad.astype(bf16).reshape(ZBLKS, 128, 2, 128).transpose(3, 0, 2, 1)
    ).reshape(128, -1)

    order = np.argsort(dst, kind="stable")
    dst_s = dst[order]
    src_s = src[order]
    win = dst_s // WIN_NODES                     # window id per sorted edge
    counts = np.bincount(win, minlength=N_WINDOWS)
    starts = np.zeros(N_WINDOWS + 1, np.int64)
    np.cumsum(counts, out=starts[1:])
    B = int(np.max((counts + 127) // 128))
    B = max(B, 1)
    EW = B * 128
    EDEV = WINS * EW

    in_maps = []
    for d in range(NDEV):
        src_pad = np.zeros((EDEV,), np.int64)
        slot_pad = np.full((EDEV,), -1.0, np.float32)
        u_pad = np.full((EDEV, TWO_MSG), 0.5, bf16)
        for k in range(WINS):
            wid = d * WINS + k
            s, e = starts[wid], starts[wid + 1]
            n = e - s
            o = k * EW
            # sort window edges by src for gather locality
            sub = np.argsort(src_s[s:e], kind="stable")
            src_pad[o : o + n] = src_s[s:e][sub]
            slot_pad[o : o + n] = (dst_s[s:e][sub] - wid * WIN_NODES).astype(np.float32)
            u_pad[o : o + n] = u2[order[s:e][sub]]

        # gather idx layout: [p, s] = idx[16*s + p%16], replicated across groups
        idx16 = np.empty((128, EDEV // 16), np.int16)
        flat = src_pad.astype(np.int16).reshape(EDEV // 16, 16).T  # [16, EDEV/16]
        for g in range(8):
            idx16[g * 16 : (g + 1) * 16, :] = flat
        # compact dstslot: [p, w*B + b] = slot of edge (w, b, p)
        dstslot_c = np.ascontiguousarray(slot_pad.reshape(WINS * B, 128).T)
        # u swizzled: [p, blk*128 + c] = u_pad[blk*128 + p, c]
        u_sw = np.ascontiguousarray(
            u_pad.reshape(EDEV // 128, 128, TWO_MSG).transpose(1, 0, 2).reshape(128, -1))
        # local featH channel-major: [p, w, a, n] = featH[2500d+125w+n, a*128+p]
        base = d * DEV_NODES
        loc = np.zeros((WINS, 128, 2, 128), np.float32)  # [w, n, a, p]
        loc[:, :WIN_NODES] = featH[base : base + DEV_NODES].reshape(
            WINS, WIN_NODES, 2, 128)
        fh_locT = np.ascontiguousarray(
            loc.astype(bf16).transpose(3, 0, 2, 1)).reshape(128, -1)
        h_loc = np.ascontiguousarray(h[base : base + DEV_NODES])
        in_maps.append({
            "fh": fh, "fh_locT": fh_locT, "h_loc": h_loc,
            "u_g": u_sw, "src16": idx16, "dstslot": dstslot_c,
        })
    return B, in_maps


def _prep_weights(W_enc, b_enc, W_dec, b_dec, W_ih, W_hh, b_ih, b_hh):
    import ml_dtypes
    bf16 = ml_dtypes.bfloat16

    W_enc = np.asarray(W_enc, np.float64)
    W_dec = np.asarray(W_dec, np.float64)
    W_ih = np.asarray(W_ih, np.float64)
    W_hh = np.asarray(W_hh, np.float64)
    b_enc = np.asarray(b_enc, np.float64)
    b_dec = np.asarray(b_dec, np.float64)
    b_ih = np.asarray(b_ih, np.float64)
    b_hh = np.asarray(b_hh, np.float64)

    W_b = W_ih[:, HIDDEN:] @ W_dec
    b_comb = W_ih[:, HIDDEN:] @ b_dec + b_ih

    # encoder difference: row j = W_enc[2j] - W_enc[2j+1]
    W_diff = W_enc[0::2] - W_enc[1::2]                       # [64, 256]
    b_diff = (b_enc[0::2] - b_enc[1::2]).astype(np.float32)  # [64]
    wdT = np.ascontiguousarray(W_diff.T).astype(bf16)        # [256, 64]
    waT = np.ascontiguousarray(W_ih[:, :HIDDEN].T).astype(bf16)   # [128, 384]
    wbT = np.ascontiguousarray(W_b.T[PERM, :]).astype(bf16)  # [128, 384] perm rows
    whhT = np.ascontiguousarray(W_hh.T).astype(bf16)         # [128, 384]
    brz = (b_comb[:256] + b_hh[:256]).astype(np.float32)
    bn = b_comb[256:384].astype(np.float32)
    bhn = b_hh[256:384].astype(np.float32)
    return {
        "wdT": wdT, "waT": waT, "wbT": wbT, "whhT": whhT,
        "bias_enc": np.ascontiguousarray(np.tile(b_diff, (128, 1))),
        "bias_rz": np.ascontiguousarray(np.tile(brz, (128, 1))),
        "bias_n": np.ascontiguousarray(np.tile(bn, (128, 1))),
        "bias_hn": np.ascontiguousarray(np.tile(bhn, (128, 1))),
    }


def kernel(feat, h, src, dst, u, W_enc, b_enc, W_dec, b_dec, W_ih, W_hh,
           b_ih, b_hh):
    B, in_maps = _prep_host(feat, h, src, dst, u)
    wmap = _prep_weights(W_enc, b_enc, W_dec, b_dec, W_ih, W_hh, b_ih, b_hh)
    for m in in_maps:
        m.update(wmap)

    phases = os.environ.get("KERNEL_PHASES", "zmg")
    zero_bias = not (np.any(np.asarray(b_enc)) or np.any(np.asarray(b_dec))
                     or np.any(np.asarray(b_ih)) or np.any(np.asarray(b_hh)))
    key = (B, phases, zero_bias)
    if key not in _cache:
        _cache[key] = build_program(B, phases, zero_bias)
    nc = _cache[key]

    res = run_bass_kernel_spmd(nc, in_maps, core_ids=list(range(NDEV)))
    h_new = np.concatenate([res.results[d]["h_new"] for d in range(NDEV)], axis=0)
    return (h_new, h_new)


# revision 10
# speedup vs baseline: 3.2129x; 1.3685x over previous
"""Trainium2 Bass kernel for nn_DiscreteCommunication (GNN message passing).

Strategy (8 NeuronCores, SPMD single program, no collectives):
  - Host: sort edges by dst; device d owns dst nodes [2500d, 2500d+2500),
    i.e. 20 windows of 125 consecutive nodes. Edges land on the device that
    owns their dst; within each window edges are sorted by src (sequential-ish
    gather addresses) and padded to B 128-edge blocks (B global so one SPMD
    program serves all cores).
  - Gumbel reformulation: m0 = argmax0 <=> d >= t2_e - t2_o where
    t2 = ln(-ln(u+eps)+eps), d = z_e - z_o. Equivalently, with
    E = exp(d) and t1 = ln(u+eps) (<= 0):  m0 = (E * t1_o <= t1_e).
    This removes the second Ln pass entirely; only one Ln per u element.
  - Device phase E: D = [feat|h] @ W_diff.T over all 20000 nodes (replicated,
    bf16 single-term matmul), E = exp(D) applied during PSUM evacuation,
    stored as bf16 [ZPAD, 128] rows (64 data + 64 zero pad -> 256B rows for
    the gather's descriptor-size constraint).
  - Device phase MSG: per window, one dma_gather of E[src] (256B rows,
    src-sorted), t1 = Ln(u+eps) from bf16 u, prod/is_le at 2x bf16 DVE rate,
    one-hot P (is_equal, split DVE/Pool), c_sumT += m.T @ P on PE per
    128-edge block, c = sign(c_sum) on ACT.
  - Device phase GRU: node-parallel bf16 GRU over the 2500 owned nodes with
    the decoder folded in: gi = feat@W_a.T + c@(W_ih[:,128:]@W_dec).T.
  - Host: concatenate the 8 per-device h_new slices.
"""
import os
import sys

sys.path.insert(0, "/opt/trn_rl_repo")

import numpy as np
import concourse.bacc as bacc
import concourse.mybir as mybir
import concourse.tile as tile
from concourse.bass_utils import run_bass_kernel_spmd

F32 = mybir.dt.float32
BF16 = mybir.dt.bfloat16
I16 = mybir.dt.int16
AF = mybir.ActivationFunctionType
OP = mybir.AluOpType

N_NODES = 20000
HIDDEN = 128
MSG = 64
TWO_MSG = 2 * MSG  # 128
N_EDGES = 320000
EPS = 1e-10
NDEV = 8
WIN_NODES = 125            # nodes per window (<=128 for one-hot slots)
WINS = 20                  # windows per device
DEV_NODES = WIN_NODES * WINS   # 2500
N_WINDOWS = NDEV * WINS        # 160, covers all 20000 nodes exactly
ZBLKS = (N_NODES + 127) // 128  # 157 blocks over nodes (last partial: 32)
ZPAD = ZBLKS * 128              # 20096

_cache = {}


def build_program(B, phases="zmg", zero_bias=True, repeats=1,
                  skip_gather=False, skip_ln=False, skip_dve=False,
                  skip_zmm=False, skip_zcopy=False, skip_zwrite=False,
                  skip_udma=False, pool_p_mod=1):
    """Build the SPMD Bass program for B blocks-per-window.
    pool_p_mod: windows with w % pool_p_mod == 1 build the one-hot P on the
    Pool engine (gpsimd) instead of DVE, to balance engine load."""
    nc = bacc.Bacc("TRN2", target_bir_lowering=False, num_swdge_queues=4)
    EW = B * 128               # padded edges per window
    EDEV = WINS * EW           # padded edges per device

    # ---- I/O ----
    # channel-major featH: [p, zb, a, n] = featH_pad[zb*128+n, a*128+p]  (bf16)
    fh = nc.dram_tensor("fh", [128, ZBLKS * 256], BF16, kind="ExternalInput")
    fh_locT = nc.dram_tensor("fh_locT", [128, WINS * 256], BF16, kind="ExternalInput")
    h_loc = nc.dram_tensor("h_loc", [DEV_NODES, HIDDEN], F32, kind="ExternalInput")
    u_g = nc.dram_tensor("u_g", [128, EDEV], BF16, kind="ExternalInput")
    src16 = nc.dram_tensor("src16", [128, EDEV // 16], I16, kind="ExternalInput")
    dstslot = nc.dram_tensor("dstslot", [128, WINS * B], F32, kind="ExternalInput")
    wdT = nc.dram_tensor("wdT", [256, MSG], BF16, kind="ExternalInput")
    waT = nc.dram_tensor("waT", [128, 384], BF16, kind="ExternalInput")
    wbT = nc.dram_tensor("wbT", [128, 384], BF16, kind="ExternalInput")
    whhT = nc.dram_tensor("whhT", [128, 384], BF16, kind="ExternalInput")
    bias_enc = nc.dram_tensor("bias_enc", [128, MSG], F32, kind="ExternalInput")
    bias_rz = nc.dram_tensor("bias_rz", [128, 256], F32, kind="ExternalInput")
    bias_n = nc.dram_tensor("bias_n", [128, 128], F32, kind="ExternalInput")
    bias_hn = nc.dram_tensor("bias_hn", [128, 128], F32, kind="ExternalInput")
    h_new = nc.dram_tensor("h_new", [DEV_NODES, HIDDEN], F32, kind="ExternalOutput")

    with tile.TileContext(nc) as tc:
        with (
            tc.tile_pool(name="const", bufs=1) as cp,
        ):
            # ---- persistent constants ----
            eps_t = cp.tile([128, 1], F32)
            nc.vector.memset(eps_t[:], EPS)
            iota_x = cp.tile([128, EW], F32)
            # values 0..127 repeated B times along free dim; exact in f32
            nc.gpsimd.iota(iota_x[:], pattern=[[0, B], [1, 128]], base=0,
                           channel_multiplier=0,
                           allow_small_or_imprecise_dtypes=True)
            dslot_t = cp.tile([128, WINS * B], F32)
            nc.sync.dma_start(out=dslot_t[:], in_=dstslot[:])
            wdT_t = cp.tile([128, 2, MSG], BF16)
            nc.sync.dma_start(out=wdT_t[:], in_=wdT.rearrange("(a p) j -> p a j", p=128))
            waT_t = cp.tile([128, 384], BF16)
            nc.sync.dma_start(out=waT_t[:], in_=waT[:])
            wbT_t = cp.tile([128, 384], BF16)
            nc.sync.dma_start(out=wbT_t[:], in_=wbT[:])
            whhT_t = cp.tile([128, 384], BF16)
            nc.sync.dma_start(out=whhT_t[:], in_=whhT[:])
            bias_enc_t = cp.tile([128, MSG], F32)
            nc.sync.dma_start(out=bias_enc_t[:], in_=bias_enc[:])
            bias_rz_t = cp.tile([128, 256], F32)
            nc.sync.dma_start(out=bias_rz_t[:], in_=bias_rz[:])
            bias_n_t = cp.tile([128, 128], F32)
            nc.sync.dma_start(out=bias_n_t[:], in_=bias_n[:])
            bias_hn_t = cp.tile([128, 128], F32)
            nc.sync.dma_start(out=bias_hn_t[:], in_=bias_hn[:])
            src16_t = cp.tile([128, EDEV // 16], I16)
            nc.sync.dma_start(out=src16_t[:], in_=src16[:])
            cT_tiles = []
            for w in range(WINS):
                ct = cp.tile([128, 128], BF16, tag=f"cT{w}")
                cT_tiles.append(ct)

            # E table: bf16 rows of 256B = [E (64 cols) | zero pad (64 cols)]
            E = nc.dram_tensor("Escr", [ZPAD, TWO_MSG], BF16)
            Ev = E.rearrange("(g p) j -> p g j", p=128)  # [128, ZBLKS, 128]

            # ---- Phase E: D = [feat|h] @ W_diff.T; E = exp(D) (all nodes) ----
            ZG = 16
            def emit_e_phase():
             with (
                tc.tile_pool(name="zio", bufs=4) as zio,
                tc.tile_pool(name="zps", bufs=2, space="PSUM") as zps,
             ):
              for g0 in range(0, ZBLKS, ZG):
                gn = min(ZG, ZBLKS - g0)
                fg = zio.tile([128, ZG, 2, 128], BF16, tag="fg")
                cols = gn * 256
                half = (cols // 2) // 128 * 128
                nc.sync.dma_start(
                    out=fg[:].rearrange("p g a n -> p (g a n)")[:, :half],
                    in_=fh[:, g0 * 256 : g0 * 256 + half])
                nc.scalar.dma_start(
                    out=fg[:].rearrange("p g a n -> p (g a n)")[:, half:cols],
                    in_=fh[:, g0 * 256 + half : g0 * 256 + cols])
                zp = zps.tile([128, ZG * MSG], F32, space="PSUM", tag="zp")
                if not skip_zmm:
                    for zi in range(gn):
                        zslc = zp[:, zi * MSG : (zi + 1) * MSG]
                        nc.tensor.matmul(out=zslc, lhsT=fg[:, zi, 0, :],
                                         rhs=wdT_t[:, 0, :], start=True, stop=False)
                        nc.tensor.matmul(out=zslc, lhsT=fg[:, zi, 1, :],
                                         rhs=wdT_t[:, 1, :], start=False, stop=True)
                else:
                    nc.vector.memset(zp[:, : gn * MSG], 0.0)
                zs = zio.tile([128, ZG, 128], BF16, tag="zs")
                # zero the pad half (bf16 2x); data half written by Exp below
                nc.vector.memset(zs[:, :gn, MSG:], 0.0)
                if skip_zcopy:
                    pass
                elif zero_bias:
                    nc.scalar.activation(
                        zs[:, :gn, :MSG],
                        zp[:, : gn * MSG].rearrange("p (g j) -> p g j", g=gn),
                        AF.Exp)
                else:
                    zb = zio.tile([128, ZG, MSG], F32, tag="zb")
                    nc.vector.tensor_tensor(
                        out=zb[:, :gn, :],
                        in0=zp[:, : gn * MSG].rearrange("p (g j) -> p g j", g=gn),
                        in1=bias_enc_t[:, None, :].to_broadcast([128, gn, MSG]),
                        op=OP.add)
                    nc.scalar.activation(zs[:, :gn, :MSG], zb[:, :gn, :], AF.Exp)
                if not skip_zwrite:
                    # SWDGE path: keeps both HWDGE rings free for featH reads
                    nc.gpsimd.dma_start(out=Ev[:, g0 : g0 + gn, :], in_=zs[:, :gn, :])

            # ---- Phase MSG + GRU, interleaved ----
            WG = 4
            def emit_msg_window(w, uw=None):
                zg = gp.tile([128, B, TWO_MSG], BF16, tag="zg")
                if not skip_gather:
                    nc.gpsimd.dma_gather(
                        zg[:], E[:],
                        src16_t[:, (w * EW) // 16 : ((w + 1) * EW) // 16],
                        num_idxs=EW, num_idxs_reg=EW, elem_size=TWO_MSG,
                        single_packet=False, queue_num=w % 4,
                    )
                else:
                    nc.gpsimd.memset(zg[:], 1.0)
                if uw is None:
                    uw = gp.tile([128, EW], BF16, tag="uw")
                    if not skip_udma:
                        ueng = nc.sync if w % 2 == 0 else nc.scalar
                        ueng.dma_start(out=uw[:], in_=u_g[:, w * EW : (w + 1) * EW])
                    else:
                        nc.gpsimd.memset(uw[:], 0.5)
                t1 = mp.tile([128, B, 128], BF16, tag="t1")
                if not skip_ln:
                    nc.scalar.activation(t1[:].rearrange("p b c -> p (b c)"), uw[:],
                                         AF.Ln, bias=eps_t[:, :1], scale=1.0)
                else:
                    nc.gpsimd.memset(t1[:], -0.7)
                m = mp.tile([128, B, 128], BF16, tag="m")
                if skip_dve:
                    nc.gpsimd.memset(m[:], 1.0)
                else:
                    prod = mp.tile([128, B, MSG], BF16, tag="prod")
                    nc.vector.tensor_tensor(out=prod[:], in0=zg[:, :, :MSG],
                                            in1=t1[:, :, MSG:], op=OP.mult)
                    nc.vector.tensor_tensor(out=m[:, :, :MSG], in0=prod[:],
                                            in1=t1[:, :, :MSG], op=OP.is_le)
                    nc.vector.tensor_scalar(out=m[:, :, MSG:], in0=m[:, :, :MSG],
                                            scalar1=1.0, scalar2=None, op0=OP.is_lt)
                P = pp.tile([128, B, 128], BF16, tag="P")
                peng = nc.gpsimd if (w % pool_p_mod == 1) else nc.vector
                peng.tensor_tensor(
                    out=P[:],
                    in0=iota_x[:].rearrange("p (b j) -> p b j", b=B),
                    in1=dslot_t[:, w * B : (w + 1) * B, None].to_broadcast([128, B, 128]),
                    op=OP.is_equal)
                cps = mps.tile([128, 128], F32, space="PSUM", tag="cps")
                for b in range(B):
                    nc.tensor.matmul(out=cps[:], lhsT=m[:, b, :], rhs=P[:, b, :],
                                     start=(b == 0), stop=(b == B - 1))
                # c = (c_sum > 0) == Sign(c_sum) since c_sum >= 0; runs on ACT
                nc.scalar.sign(out=cT_tiles[w][:], in_=cps[:])

            def emit_gru_group(w0):
                xh = rp.tile([128, WG, 2, 128], BF16, tag="xh")
                nc.sync.dma_start(
                    out=xh[:].rearrange("p w a n -> p (w a n)"),
                    in_=fh_locT[:, w0 * 256 : (w0 + WG) * 256])
                hl = rp.tile([128, WG, 128], F32, tag="hl")
                for wi in range(WG):
                    w = w0 + wi
                    nc.sync.dma_start(
                        out=hl[:WIN_NODES, wi, :],
                        in_=h_loc[w * WIN_NODES : (w + 1) * WIN_NODES, :])
                # pad each window's slice to 512 f32 = one full PSUM bank so no
                # matmul output crosses a bank boundary
                gi = rps.tile([128, WG, 512], F32, space="PSUM", tag="gi")
                hn_ps = rps2.tile([128, WG, 128], F32, space="PSUM", tag="hn_ps")
                for wi in range(WG):
                    w = w0 + wi
                    nc.tensor.matmul(out=gi[:, wi, 0:384], lhsT=xh[:, wi, 0, :],
                                     rhs=waT_t[:], start=True, stop=False)
                    nc.tensor.matmul(out=gi[:, wi, 0:384], lhsT=cT_tiles[w][:],
                                     rhs=wbT_t[:], start=False, stop=False)
                    nc.tensor.matmul(out=gi[:, wi, 0:256], lhsT=xh[:, wi, 1, :],
                                     rhs=whhT_t[:, 0:256], start=False, stop=True,
                                     skip_group_check=True)
                    nc.tensor.matmul(out=hn_ps[:, wi, :], lhsT=xh[:, wi, 1, :],
                                     rhs=whhT_t[:, 256:384], start=True, stop=True)
                rz_s = rp.tile([128, WG, 256], F32, tag="rz_s")
                if zero_bias:
                    nc.scalar.activation(rz_s[:], gi[:, :, 0:256], AF.Sigmoid)
                    rhn = rp.tile([128, WG, 128], F32, tag="rhn")
                    nc.vector.tensor_tensor(out=rhn[:], in0=rz_s[:, :, 0:128],
                                            in1=hn_ps[:], op=OP.mult)
                    narg = rp.tile([128, WG, 128], F32, tag="narg")
                    nc.vector.tensor_tensor(out=narg[:], in0=rhn[:],
                                            in1=gi[:, :, 256:384], op=OP.add)
                else:
                    rz = rp.tile([128, WG, 256], F32, tag="rz")
                    nc.vector.tensor_tensor(
                        out=rz[:], in0=gi[:, :, 0:256],
                        in1=bias_rz_t[:, None, :].to_broadcast([128, WG, 256]), op=OP.add)
                    nc.scalar.activation(rz_s[:], rz[:], AF.Sigmoid)
                    hn = rp.tile([128, WG, 128], F32, tag="hn")
                    nc.vector.tensor_tensor(
                        out=hn[:], in0=hn_ps[:],
                        in1=bias_hn_t[:, None, :].to_broadcast([128, WG, 128]), op=OP.add)
                    inn = rp.tile([128, WG, 128], F32, tag="inn")
                    nc.vector.tensor_tensor(
                        out=inn[:], in0=gi[:, :, 256:384],
                        in1=bias_n_t[:, None, :].to_broadcast([128, WG, 128]), op=OP.add)
                    rhn = rp.tile([128, WG, 128], F32, tag="rhn")
                    nc.vector.tensor_tensor(out=rhn[:], in0=rz_s[:, :, 0:128], in1=hn[:], op=OP.mult)
                    narg = rp.tile([128, WG, 128], F32, tag="narg")
                    nc.vector.tensor_tensor(out=narg[:], in0=inn[:], in1=rhn[:], op=OP.add)
                n_t = rp.tile([128, WG, 128], F32, tag="n_t")
                nc.scalar.activation(n_t[:], narg[:], AF.Tanh)
                hmn = rp.tile([128, WG, 128], F32, tag="hmn")
                nc.vector.tensor_tensor(out=hmn[:], in0=hl[:], in1=n_t[:], op=OP.subtract)
                zh = rp.tile([128, WG, 128], F32, tag="zh")
                nc.vector.tensor_tensor(out=zh[:], in0=rz_s[:, :, 128:256], in1=hmn[:], op=OP.mult)
                ho = rp.tile([128, WG, 128], F32, tag="ho")
                nc.vector.tensor_tensor(out=ho[:], in0=n_t[:], in1=zh[:], op=OP.add)
                for wi in range(WG):
                    w = w0 + wi
                    nc.sync.dma_start(
                        out=h_new[w * WIN_NODES : (w + 1) * WIN_NODES, :],
                        in_=ho[:WIN_NODES, wi, :])

            for _rep in range(repeats):
                with (
                    tc.tile_pool(name="msg", bufs=3) as mp,
                    tc.tile_pool(name="ponehot", bufs=3) as pp,
                    tc.tile_pool(name="gat", bufs=3) as gp,
                    tc.tile_pool(name="mps", bufs=2, space="PSUM") as mps,
                ):
                    # prefetch u for the first windows so HWDGE rings stay
                    # busy while phase E owns PE/ACT
                    uw_pre = {}
                    if "m" in phases and not skip_udma:
                        for w in range(min(2, WINS)):
                            uw = gp.tile([128, EW], BF16, tag="uw")
                            ueng = nc.sync if w % 2 == 0 else nc.scalar
                            ueng.dma_start(out=uw[:],
                                           in_=u_g[:, w * EW : (w + 1) * EW])
                            uw_pre[w] = uw
                    if "z" in phases:
                        emit_e_phase()
                    for w in range(WINS):
                        if "m" in phases:
                            emit_msg_window(w, uw_pre.pop(w, None))
                    if "g" in phases:
                        with (
                            tc.tile_pool(name="gru", bufs=2) as rp,
                            tc.tile_pool(name="rps", bufs=1, space="PSUM") as rps,
                            tc.tile_pool(name="rps2", bufs=2, space="PSUM") as rps2,
                        ):
                            for w0 in range(0, WINS, WG):
                                emit_gru_group(w0)

    nc.compile()
    return nc


# message-column permutation: evens first, then odds
PERM = np.concatenate([np.arange(0, TWO_MSG, 2), np.arange(1, TWO_MSG, 2)])


def _prep_host(feat, h, src, dst, u):
    """Host-side sharding/layout. Returns (B, list of per-core in_maps)."""
    import ml_dtypes
    bf16 = ml_dtypes.bfloat16

    feat = np.ascontiguousarray(feat, dtype=np.float32)
    h = np.ascontiguousarray(h, dtype=np.float32)
    src = np.asarray(src).astype(np.int64)
    dst = np.asarray(dst).astype(np.int64)
    u2 = np.ascontiguousarray(u, dtype=np.float32).reshape(N_EDGES, TWO_MSG)
    u2 = u2[:, PERM].astype(bf16)

    featH = np.concatenate([feat, h], axis=1)  # [N, 256]
    featH_pad = np.zeros((ZPAD, 256), np.float32)
    featH_pad[:N_NODES] = featH

    # channel-major: [p, zb, a, n] = x[zb*128+n, a*128+p]
    fh = np.ascontiguousarray(
        featH_pad.astype(bf16).reshape(ZBLKS, 128, 2, 128).transpose(3, 0, 2, 1)
    ).reshape(128, -1)

    order = np.argsort(dst, kind="stable")
    dst_s = dst[order]
    src_s = src[order]
    win = dst_s // WIN_NODES                     # window id per sorted edge
    counts = np.bincount(win, minlength=N_WINDOWS)
    starts = np.zeros(N_WINDOWS + 1, np.int64)
    np.cumsum(counts, out=starts[1:])
    B = int(np.max((counts + 127) // 128))
    B = max(B, 1)
    EW = B * 128
    EDEV = WINS * EW

    in_maps = []
    for d in range(NDEV):
        src_pad = np.zeros((EDEV,), np.int64)
        slot_pad = np.full((EDEV,), -1.0, np.float32)
        u_pad = np.full((EDEV, TWO_MSG), 0.5, bf16)
        for k in range(WINS):
            wid = d * WINS + k
            s, e = starts[wid], starts[wid + 1]
            n = e - s
            o = k * EW
            # sort window edges by src for gather locality
            sub = np.argsort(src_s[s:e], kind="stable")
            src_pad[o : o + n] = src_s[s:e][sub]
            slot_pad[o : o + n] = (dst_s[s:e][sub] - wid * WIN_NODES).astype(np.float32)
            u_pad[o : o + n] = u2[order[s:e][sub]]

        # gather idx layout: [p, s] = idx[16*s + p%16], replicated across groups
        idx16 = np.empty((128, EDEV // 16), np.int16)
        flat = src_pad.astype(np.int16).reshape(EDEV // 16, 16).T  # [16, EDEV/16]
        for g in range(8):
            idx16[g * 16 : (g + 1) * 16, :] = flat
        # compact dstslot: [p, w*B + b] = slot of edge (w, b, p)
        dstslot_c = np.ascontiguousarray(slot_pad.reshape(WINS * B, 128).T)
        # u swizzled: [p, blk*128 + c] = u_pad[blk*128 + p, c]
        u_sw = np.ascontiguousarray(
            u_pad.reshape(EDEV // 128, 128, TWO_MSG).transpose(1, 0, 2).reshape(128, -1))
        # local featH channel-major: [p, w, a, n] = featH[2500d+125w+n, a*128+p]
        base = d * DEV_NODES
        loc = np.zeros((WINS, 128, 2, 128), np.float32)  # [w, n, a, p]
        loc[:, :WIN_NODES] = featH[base : base + DEV_NODES].reshape(
            WINS, WIN_NODES, 2, 128)
        fh_locT = np.ascontiguousarray(
            loc.astype(bf16).transpose(3, 0, 2, 1)).reshape(128, -1)
        h_loc = np.ascontiguousarray(h[base : base + DEV_NODES])
        in_maps.append({
            "fh": fh, "fh_locT": fh_locT, "h_loc": h_loc,
            "u_g": u_sw, "src16": idx16, "dstslot": dstslot_c,
        })
    return B, in_maps


def _prep_weights(W_enc, b_enc, W_dec, b_dec, W_ih, W_hh, b_ih, b_hh):
    import ml_dtypes
    bf16 = ml_dtypes.bfloat16

    W_enc = np.asarray(W_enc, np.float64)
    W_dec = np.asarray(W_dec, np.float64)
    W_ih = np.asarray(W_ih, np.float64)
    W_hh = np.asarray(W_hh, np.float64)
    b_enc = np.asarray(b_enc, np.float64)
    b_dec = np.asarray(b_dec, np.float64)
    b_ih = np.asarray(b_ih, np.float64)
    b_hh = np.asarray(b_hh, np.float64)

    W_b = W_ih[:, HIDDEN:] @ W_dec
    b_comb = W_ih[:, HIDDEN:] @ b_dec + b_ih

    # encoder difference: row j = W_enc[2j] - W_enc[2j+1]
    W_diff = W_enc[0::2] - W_enc[1::2]                       # [64, 256]
    b_diff = (b_enc[0::2] - b_enc[1::2]).astype(np.float32)  # [64]
    wdT = np.ascontiguousarray(W_diff.T).astype(bf16)        # [256, 64]
    waT = np.ascontiguousarray(W_ih[:, :HIDDEN].T).astype(bf16)   # [128, 384]
    wbT = np.ascontiguousarray(W_b.T[PERM, :]).astype(bf16)  # [128, 384] perm rows
    whhT = np.ascontiguousarray(W_hh.T).astype(bf16)         # [128, 384]
    brz = (b_comb[:256] + b_hh[:256]).astype(np.float32)
    bn = b_comb[256:384].astype(np.float32)
    bhn = b_hh[256:384].astype(np.float32)
    return {
        "wdT": wdT, "waT": waT, "wbT": wbT, "whhT": whhT,
        "bias_enc": np.ascontiguousarray(np.tile(b_diff, (128, 1))),
        "bias_rz": np.ascontiguousarray(np.tile(brz, (128, 1))),
        "bias_n": np.ascontiguousarray(np.tile(bn, (128, 1))),
        "bias_hn": np.ascontiguousarray(np.tile(bhn, (128, 1))),
    }


def kernel(feat, h, src, dst, u, W_enc, b_enc, W_dec, b_dec, W_ih, W_hh,
           b_ih, b_hh):
    B, in_maps = _prep_host(feat, h, src, dst, u)
    wmap = _prep_weights(W_enc, b_enc, W_dec, b_dec, W_ih, W_hh, b_ih, b_hh)
    for m in in_maps:
        m.update(wmap)

    phases = os.environ.get("KERNEL_PHASES", "zmg")
    zero_bias = not (np.any(np.asarray(b_enc)) or np.any(np.asarray(b_dec))
                     or np.any(np.asarray(b_ih)) or np.any(np.asarray(b_hh)))
    key = (B, phases, zero_bias)
    if key not in _cache:
        _cache[key] = build_program(B, phases, zero_bias)
    nc = _cache[key]

    res = run_bass_kernel_spmd(nc, in_maps, core_ids=list(range(NDEV)))
    h_new = np.concatenate([res.results[d]["h_new"] for d in range(NDEV)], axis=0)
    return (h_new, h_new)


# revision 12
# speedup vs baseline: 4.0869x; 1.2720x over previous
"""Trainium2 Bass kernel for nn_DiscreteCommunication (GNN message passing).

Strategy (8 NeuronCores, SPMD single program, no collectives):
  - Host: sort edges by dst; device d owns dst nodes [2500d, 2500d+2500),
    i.e. 20 windows of 125 consecutive nodes. Edges land on the device that
    owns their dst; within each window edges are sorted by src (sequential-ish
    gather addresses) and padded to B 128-edge blocks (B global so one SPMD
    program serves all cores).
  - Gumbel reformulation: m0 = argmax0 <=> d >= t2_e - t2_o where
    t2 = ln(-ln(u+eps)+eps), d = z_e - z_o. Equivalently, with
    E = exp(d) and t1 = ln(u+eps) (<= 0):  m0 = (E * t1_o <= t1_e).
    This removes the second Ln pass entirely; only one Ln per u element.
  - Device phase E: D = [feat|h] @ W_diff.T over all 20000 nodes (replicated,
    bf16 single-term matmul), E = exp(D) applied during PSUM evacuation,
    stored as bf16 [ZPAD, 128] rows (64 data + 64 zero pad -> 256B rows for
    the gather's descriptor-size constraint).
  - Device phase MSG: per window, one dma_gather of E[src] (256B rows,
    src-sorted), t1 = Ln(u+eps) from bf16 u, prod/is_le at 2x bf16 DVE rate,
    one-hot P (is_equal, split DVE/Pool), c_sumT += m.T @ P on PE per
    128-edge block, c = sign(c_sum) on ACT.
  - Device phase GRU: node-parallel bf16 GRU over the 2500 owned nodes with
    the decoder folded in: gi = feat@W_a.T + c@(W_ih[:,128:]@W_dec).T.
  - Host: concatenate the 8 per-device h_new slices.
"""
import os
import sys

sys.path.insert(0, "/opt/trn_rl_repo")

import numpy as np
import concourse.bacc as bacc
import concourse.mybir as mybir
import concourse.tile as tile
from concourse.bass_utils import run_bass_kernel_spmd

F32 = mybir.dt.float32
BF16 = mybir.dt.bfloat16
I16 = mybir.dt.int16
AF = mybir.ActivationFunctionType
OP = mybir.AluOpType

N_NODES = 20000
HIDDEN = 128
MSG = 64
TWO_MSG = 2 * MSG  # 128
N_EDGES = 320000
EPS = 1e-10
NDEV = 8
WIN_NODES = 125            # nodes per window (<=128 for one-hot slots)
WINS = 20                  # windows per device
DEV_NODES = WIN_NODES * WINS   # 2500
N_WINDOWS = NDEV * WINS        # 160, covers all 20000 nodes exactly
ZBLKS = (N_NODES + 127) // 128  # 157 blocks over nodes (last partial: 32)
ZPAD = ZBLKS * 128              # 20096

_cache = {}


def build_program(B, phases="zmg", zero_bias=True, repeats=1,
                  skip_gather=False, skip_ln=False, skip_dve=False,
                  skip_zmm=False, skip_zcopy=False, skip_zwrite=False,
                  skip_udma=False, pool_p_mod=1):
    """Build the SPMD Bass program for B blocks-per-window.
    pool_p_mod: windows with w % pool_p_mod == 1 build the one-hot P on the
    Pool engine (gpsimd) instead of DVE, to balance engine load."""
    nc = bacc.Bacc("TRN2", target_bir_lowering=False, num_swdge_queues=4)
    EW = B * 128               # padded edges per window
    EDEV = WINS * EW           # padded edges per device

    # ---- I/O ----
    # channel-major featH: [p, zb, a, n] = featH_pad[zb*128+n, a*128+p]  (bf16)
    fh = nc.dram_tensor("fh", [128, ZBLKS * 256], BF16, kind="ExternalInput")
    fh_locT = nc.dram_tensor("fh_locT", [128, WINS * 256], BF16, kind="ExternalInput")
    h_loc = nc.dram_tensor("h_loc", [DEV_NODES, HIDDEN], F32, kind="ExternalInput")
    u_g = nc.dram_tensor("u_g", [128, EDEV], BF16, kind="ExternalInput")
    src16 = nc.dram_tensor("src16", [128, EDEV // 16], I16, kind="ExternalInput")
    dstslot = nc.dram_tensor("dstslot", [128, WINS * B], F32, kind="ExternalInput")
    wdT = nc.dram_tensor("wdT", [256, MSG], BF16, kind="ExternalInput")
    waT = nc.dram_tensor("waT", [128, 384], BF16, kind="ExternalInput")
    wbT = nc.dram_tensor("wbT", [128, 384], BF16, kind="ExternalInput")
    whhT = nc.dram_tensor("whhT", [128, 384], BF16, kind="ExternalInput")
    bias_enc = nc.dram_tensor("bias_enc", [128, MSG], F32, kind="ExternalInput")
    bias_rz = nc.dram_tensor("bias_rz", [128, 256], F32, kind="ExternalInput")
    bias_n = nc.dram_tensor("bias_n", [128, 128], F32, kind="ExternalInput")
    bias_hn = nc.dram_tensor("bias_hn", [128, 128], F32, kind="ExternalInput")
    h_new = nc.dram_tensor("h_new", [DEV_NODES, HIDDEN], F32, kind="ExternalOutput")

    with tile.TileContext(nc) as tc:
        with (
            tc.tile_pool(name="const", bufs=1) as cp,
        ):
            # ---- persistent constants ----
            eps_t = cp.tile([128, 1], F32)
            nc.vector.memset(eps_t[:], EPS)
            iota_x = cp.tile([128, EW], F32)
            # values 0..127 repeated B times along free dim; exact in f32
            nc.gpsimd.iota(iota_x[:], pattern=[[0, B], [1, 128]], base=0,
                           channel_multiplier=0,
                           allow_small_or_imprecise_dtypes=True)
            dslot_t = cp.tile([128, WINS * B], F32)
            nc.sync.dma_start(out=dslot_t[:], in_=dstslot[:])
            wdT_t = cp.tile([128, 2, MSG], BF16)
            nc.sync.dma_start(out=wdT_t[:], in_=wdT.rearrange("(a p) j -> p a j", p=128))
            waT_t = cp.tile([128, 384], BF16)
            nc.sync.dma_start(out=waT_t[:], in_=waT[:])
            wbT_t = cp.tile([128, 384], BF16)
            nc.sync.dma_start(out=wbT_t[:], in_=wbT[:])
            whhT_t = cp.tile([128, 384], BF16)
            nc.sync.dma_start(out=whhT_t[:], in_=whhT[:])
            bias_enc_t = cp.tile([128, MSG], F32)
            nc.sync.dma_start(out=bias_enc_t[:], in_=bias_enc[:])
            bias_rz_t = cp.tile([128, 256], F32)
            nc.sync.dma_start(out=bias_rz_t[:], in_=bias_rz[:])
            bias_n_t = cp.tile([128, 128], F32)
            nc.sync.dma_start(out=bias_n_t[:], in_=bias_n[:])
            bias_hn_t = cp.tile([128, 128], F32)
            nc.sync.dma_start(out=bias_hn_t[:], in_=bias_hn[:])
            src16_t = cp.tile([128, EDEV // 16], I16)
            nc.sync.dma_start(out=src16_t[:], in_=src16[:])
            cT_tiles = []
            for w in range(WINS):
                ct = cp.tile([128, 128], BF16, tag=f"cT{w}")
                cT_tiles.append(ct)

            # E table: bf16 rows of 256B = [E (64 cols) | zero pad (64 cols)]
            E = nc.dram_tensor("Escr", [ZPAD, TWO_MSG], BF16)
            Ev = E.rearrange("(g p) j -> p g j", p=128)  # [128, ZBLKS, 128]

            # ---- Phase E: D = [feat|h] @ W_diff.T; E = exp(D) (all nodes) ----
            ZG = 16
            def emit_e_phase():
             with (
                tc.tile_pool(name="zio", bufs=4) as zio,
                tc.tile_pool(name="zps", bufs=2, space="PSUM") as zps,
             ):
              for g0 in range(0, ZBLKS, ZG):
                gn = min(ZG, ZBLKS - g0)
                fg = zio.tile([128, ZG, 2, 128], BF16, tag="fg")
                cols = gn * 256
                half = (cols // 2) // 128 * 128
                nc.sync.dma_start(
                    out=fg[:].rearrange("p g a n -> p (g a n)")[:, :half],
                    in_=fh[:, g0 * 256 : g0 * 256 + half])
                nc.scalar.dma_start(
                    out=fg[:].rearrange("p g a n -> p (g a n)")[:, half:cols],
                    in_=fh[:, g0 * 256 + half : g0 * 256 + cols])
                zp = zps.tile([128, ZG * MSG], F32, space="PSUM", tag="zp")
                if not skip_zmm:
                    for zi in range(gn):
                        zslc = zp[:, zi * MSG : (zi + 1) * MSG]
                        nc.tensor.matmul(out=zslc, lhsT=fg[:, zi, 0, :],
                                         rhs=wdT_t[:, 0, :], start=True, stop=False)
                        nc.tensor.matmul(out=zslc, lhsT=fg[:, zi, 1, :],
                                         rhs=wdT_t[:, 1, :], start=False, stop=True)
                else:
                    nc.vector.memset(zp[:, : gn * MSG], 0.0)
                zs = zio.tile([128, ZG, 128], BF16, tag="zs")
                # zero the pad half (bf16 2x); data half written by Exp below
                nc.vector.memset(zs[:, :gn, MSG:], 0.0)
                if skip_zcopy:
                    pass
                elif zero_bias:
                    nc.scalar.activation(
                        zs[:, :gn, :MSG],
                        zp[:, : gn * MSG].rearrange("p (g j) -> p g j", g=gn),
                        AF.Exp)
                else:
                    zb = zio.tile([128, ZG, MSG], F32, tag="zb")
                    nc.vector.tensor_tensor(
                        out=zb[:, :gn, :],
                        in0=zp[:, : gn * MSG].rearrange("p (g j) -> p g j", g=gn),
                        in1=bias_enc_t[:, None, :].to_broadcast([128, gn, MSG]),
                        op=OP.add)
                    nc.scalar.activation(zs[:, :gn, :MSG], zb[:, :gn, :], AF.Exp)
                if not skip_zwrite:
                    # SWDGE path: keeps both HWDGE rings free for featH reads
                    nc.gpsimd.dma_start(out=Ev[:, g0 : g0 + gn, :], in_=zs[:, :gn, :])

            # ---- Phase MSG + GRU, interleaved ----
            WG = 4
            def emit_msg_window(w, uw=None):
                zg = gp.tile([128, B, TWO_MSG], BF16, tag="zg")
                if not skip_gather:
                    # split across two SWDGE queues so two queues work on the
                    # same window concurrently (4 queues = 2 windows in flight)
                    hb = (B // 2) * 128
                    for ci, (o0, o1) in enumerate(((0, hb), (hb, EW))):
                        nc.gpsimd.dma_gather(
                            zg[:, o0 // 128 : o1 // 128, :], E[:],
                            src16_t[:, (w * EW + o0) // 16 : (w * EW + o1) // 16],
                            num_idxs=o1 - o0, num_idxs_reg=o1 - o0,
                            elem_size=TWO_MSG, single_packet=False,
                            queue_num=(2 * w + ci) % 4,
                        )
                else:
                    nc.gpsimd.memset(zg[:], 1.0)
                if uw is None:
                    uw = gp.tile([128, EW], BF16, tag="uw")
                    if not skip_udma:
                        ueng = nc.sync if w % 2 == 0 else nc.scalar
                        ueng.dma_start(out=uw[:], in_=u_g[:, w * EW : (w + 1) * EW])
                    else:
                        nc.gpsimd.memset(uw[:], 0.5)
                t1 = mp.tile([128, B, 128], BF16, tag="t1")
                if not skip_ln:
                    nc.scalar.activation(t1[:].rearrange("p b c -> p (b c)"), uw[:],
                                         AF.Ln, bias=eps_t[:, :1], scale=1.0)
                else:
                    nc.gpsimd.memset(t1[:], -0.7)
                m = mp.tile([128, B, 128], BF16, tag="m")
                if skip_dve:
                    nc.gpsimd.memset(m[:], 1.0)
                else:
                    prod = mp.tile([128, B, MSG], BF16, tag="prod")
                    nc.vector.tensor_tensor(out=prod[:], in0=zg[:, :, :MSG],
                                            in1=t1[:, :, MSG:], op=OP.mult)
                    nc.vector.tensor_tensor(out=m[:, :, :MSG], in0=prod[:],
                                            in1=t1[:, :, :MSG], op=OP.is_le)
                    nc.vector.tensor_scalar(out=m[:, :, MSG:], in0=m[:, :, :MSG],
                                            scalar1=1.0, scalar2=None, op0=OP.is_lt)
                P = pp.tile([128, B, 128], BF16, tag="P")
                peng = nc.gpsimd if (w % pool_p_mod == 1) else nc.vector
                peng.tensor_tensor(
                    out=P[:],
                    in0=iota_x[:].rearrange("p (b j) -> p b j", b=B),
                    in1=dslot_t[:, w * B : (w + 1) * B, None].to_broadcast([128, B, 128]),
                    op=OP.is_equal)
                cps = mps.tile([128, 128], F32, space="PSUM", tag="cps")
                for b in range(B):
                    nc.tensor.matmul(out=cps[:], lhsT=m[:, b, :], rhs=P[:, b, :],
                                     start=(b == 0), stop=(b == B - 1))
                # c = (c_sum > 0) == Sign(c_sum) since c_sum >= 0; runs on ACT
                nc.scalar.sign(out=cT_tiles[w][:], in_=cps[:])

            def emit_gru_group(w0):
                xh = rp.tile([128, WG, 2, 128], BF16, tag="xh")
                nc.sync.dma_start(
                    out=xh[:].rearrange("p w a n -> p (w a n)"),
                    in_=fh_locT[:, w0 * 256 : (w0 + WG) * 256])
                hl = rp.tile([128, WG, 128], F32, tag="hl")
                for wi in range(WG):
                    w = w0 + wi
                    nc.sync.dma_start(
                        out=hl[:WIN_NODES, wi, :],
                        in_=h_loc[w * WIN_NODES : (w + 1) * WIN_NODES, :])
                # pad each window's slice to 512 f32 = one full PSUM bank so no
                # matmul output crosses a bank boundary
                gi = rps.tile([128, WG, 512], F32, space="PSUM", tag="gi")
                hn_ps = rps2.tile([128, WG, 128], F32, space="PSUM", tag="hn_ps")
                for wi in range(WG):
                    w = w0 + wi
                    nc.tensor.matmul(out=gi[:, wi, 0:384], lhsT=xh[:, wi, 0, :],
                                     rhs=waT_t[:], start=True, stop=False)
                    nc.tensor.matmul(out=gi[:, wi, 0:384], lhsT=cT_tiles[w][:],
                                     rhs=wbT_t[:], start=False, stop=False)
                    nc.tensor.matmul(out=gi[:, wi, 0:256], lhsT=xh[:, wi, 1, :],
                                     rhs=whhT_t[:, 0:256], start=False, stop=True,
                                     skip_group_check=True)
                    nc.tensor.matmul(out=hn_ps[:, wi, :], lhsT=xh[:, wi, 1, :],
                                     rhs=whhT_t[:, 256:384], start=True, stop=True)
                rz_s = rp.tile([128, WG, 256], F32, tag="rz_s")
                if zero_bias:
                    nc.scalar.activation(rz_s[:], gi[:, :, 0:256], AF.Sigmoid)
                    rhn = rp.tile([128, WG, 128], F32, tag="rhn")
                    nc.vector.tensor_tensor(out=rhn[:], in0=rz_s[:, :, 0:128],
                                            in1=hn_ps[:], op=OP.mult)
                    narg = rp.tile([128, WG, 128], F32, tag="narg")
                    nc.vector.tensor_tensor(out=narg[:], in0=rhn[:],
                                            in1=gi[:, :, 256:384], op=OP.add)
                else:
                    rz = rp.tile([128, WG, 256], F32, tag="rz")
                    nc.vector.tensor_tensor(
                        out=rz[:], in0=gi[:, :, 0:256],
                        in1=bias_rz_t[:, None, :].to_broadcast([128, WG, 256]), op=OP.add)
                    nc.scalar.activation(rz_s[:], rz[:], AF.Sigmoid)
                    hn = rp.tile([128, WG, 128], F32, tag="hn")
                    nc.vector.tensor_tensor(
                        out=hn[:], in0=hn_ps[:],
                        in1=bias_hn_t[:, None, :].to_broadcast([128, WG, 128]), op=OP.add)
                    inn = rp.tile([128, WG, 128], F32, tag="inn")
                    nc.vector.tensor_tensor(
                        out=inn[:], in0=gi[:, :, 256:384],
                        in1=bias_n_t[:, None, :].to_broadcast([128, WG, 128]), op=OP.add)
                    rhn = rp.tile([128, WG, 128], F32, tag="rhn")
                    nc.vector.tensor_tensor(out=rhn[:], in0=rz_s[:, :, 0:128], in1=hn[:], op=OP.mult)
                    narg = rp.tile([128, WG, 128], F32, tag="narg")
                    nc.vector.tensor_tensor(out=narg[:], in0=inn[:], in1=rhn[:], op=OP.add)
                n_t = rp.tile([128, WG, 128], F32, tag="n_t")
                nc.scalar.activation(n_t[:], narg[:], AF.Tanh)
                hmn = rp.tile([128, WG, 128], F32, tag="hmn")
                nc.vector.tensor_tensor(out=hmn[:], in0=hl[:], in1=n_t[:], op=OP.subtract)
                zh = rp.tile([128, WG, 128], F32, tag="zh")
                nc.vector.tensor_tensor(out=zh[:], in0=rz_s[:, :, 128:256], in1=hmn[:], op=OP.mult)
                ho = rp.tile([128, WG, 128], F32, tag="ho")
                nc.vector.tensor_tensor(out=ho[:], in0=n_t[:], in1=zh[:], op=OP.add)
                for wi in range(WG):
                    w = w0 + wi
                    nc.sync.dma_start(
                        out=h_new[w * WIN_NODES : (w + 1) * WIN_NODES, :],
                        in_=ho[:WIN_NODES, wi, :])

            for _rep in range(repeats):
                with (
                    tc.tile_pool(name="msg", bufs=3) as mp,
                    tc.tile_pool(name="ponehot", bufs=3) as pp,
                    tc.tile_pool(name="gat", bufs=5) as gp,
                    tc.tile_pool(name="mps", bufs=2, space="PSUM") as mps,
                ):
                    # prefetch u for the first windows so HWDGE rings stay
                    # busy while phase E owns PE/ACT
                    uw_pre = {}
                    if "m" in phases and not skip_udma:
                        for w in range(min(2, WINS)):
                            uw = gp.tile([128, EW], BF16, tag="uw")
                            ueng = nc.sync if w % 2 == 0 else nc.scalar
                            ueng.dma_start(out=uw[:],
                                           in_=u_g[:, w * EW : (w + 1) * EW])
                            uw_pre[w] = uw
                    if "z" in phases:
                        emit_e_phase()
                    for w in range(WINS):
                        if "m" in phases:
                            emit_msg_window(w, uw_pre.pop(w, None))
                    if "g" in phases:
                        with (
                            tc.tile_pool(name="gru", bufs=2) as rp,
                            tc.tile_pool(name="rps", bufs=1, space="PSUM") as rps,
                            tc.tile_pool(name="rps2", bufs=2, space="PSUM") as rps2,
                        ):
                            for w0 in range(0, WINS, WG):
                                emit_gru_group(w0)

    nc.compile()
    return nc


# message-column permutation: evens first, then odds
PERM = np.concatenate([np.arange(0, TWO_MSG, 2), np.arange(1, TWO_MSG, 2)])


def _prep_host(feat, h, src, dst, u):
    """Host-side sharding/layout. Returns (B, list of per-core in_maps)."""
    import ml_dtypes
    bf16 = ml_dtypes.bfloat16

    feat = np.ascontiguousarray(feat, dtype=np.float32)
    h = np.ascontiguousarray(h, dtype=np.float32)
    src = np.asarray(src).astype(np.int64)
    dst = np.asarray(dst).astype(np.int64)
    u2 = np.ascontiguousarray(u, dtype=np.float32).reshape(N_EDGES, TWO_MSG)
    u2 = u2[:, PERM].astype(bf16)

    featH = np.concatenate([feat, h], axis=1)  # [N, 256]
    featH_pad = np.zeros((ZPAD, 256), np.float32)
    featH_pad[:N_NODES] = featH

    # channel-major: [p, zb, a, n] = x[zb*128+n, a*128+p]
    fh = np.ascontiguousarray(
        featH_pad.astype(bf16).reshape(ZBLKS, 128, 2, 128).transpose(3, 0, 2, 1)
    ).reshape(128, -1)

    order = np.argsort(dst, kind="stable")
    dst_s = dst[order]
    src_s = src[order]
    win = dst_s // WIN_NODES                     # window id per sorted edge
    counts = np.bincount(win, minlength=N_WINDOWS)
    starts = np.zeros(N_WINDOWS + 1, np.int64)
    np.cumsum(counts, out=starts[1:])
    B = int(np.max((counts + 127) // 128))
    B = max(B, 1)
    EW = B * 128
    EDEV = WINS * EW

    in_maps = []
    for d in range(NDEV):
        src_pad = np.zeros((EDEV,), np.int64)
        slot_pad = np.full((EDEV,), -1.0, np.float32)
        u_pad = np.full((EDEV, TWO_MSG), 0.5, bf16)
        for k in range(WINS):
            wid = d * WINS + k
            s, e = starts[wid], starts[wid + 1]
            n = e - s
            o = k * EW
            # sort window edges by src for gather locality
            sub = np.argsort(src_s[s:e], kind="stable")
            src_pad[o : o + n] = src_s[s:e][sub]
            slot_pad[o : o + n] = (dst_s[s:e][sub] - wid * WIN_NODES).astype(np.float32)
            u_pad[o : o + n] = u2[order[s:e][sub]]

        # gather idx layout: [p, s] = idx[16*s + p%16], replicated across groups
        idx16 = np.empty((128, EDEV // 16), np.int16)
        flat = src_pad.astype(np.int16).reshape(EDEV // 16, 16).T  # [16, EDEV/16]
        for g in range(8):
            idx16[g * 16 : (g + 1) * 16, :] = flat
        # compact dstslot: [p, w*B + b] = slot of edge (w, b, p)
        dstslot_c = np.ascontiguousarray(slot_pad.reshape(WINS * B, 128).T)
        # u swizzled: [p, blk*128 + c] = u_pad[blk*128 + p, c]
        u_sw = np.ascontiguousarray(
            u_pad.reshape(EDEV // 128, 128, TWO_MSG).transpose(1, 0, 2).reshape(128, -1))
        # local featH channel-major: [p, w, a, n] = featH[2500d+125w+n, a*128+p]
        base = d * DEV_NODES
        loc = np.zeros((WINS, 128, 2, 128), np.float32)  # [w, n, a, p]
        loc[:, :WIN_NODES] = featH[base : base + DEV_NODES].reshape(
            WINS, WIN_NODES, 2, 128)
        fh_locT = np.ascontiguousarray(
            loc.astype(bf16).transpose(3, 0, 2, 1)).reshape(128, -1)
        h_loc = np.ascontiguousarray(h[base : base + DEV_NODES])
        in_maps.append({
            "fh": fh, "fh_locT": fh_locT, "h_loc": h_loc,
            "u_g": u_sw, "src16": idx16, "dstslot": dstslot_c,
        })
    return B, in_maps


def _prep_weights(W_enc, b_enc, W_dec, b_dec, W_ih, W_hh, b_ih, b_hh):
    import ml_dtypes
    bf16 = ml_dtypes.bfloat16

    W_enc = np.asarray(W_enc, np.float64)
    W_dec = np.asarray(W_dec, np.float64)
    W_ih = np.asarray(W_ih, np.float64)
    W_hh = np.asarray(W_hh, np.float64)
    b_enc = np.asarray(b_enc, np.float64)
    b_dec = np.asarray(b_dec, np.float64)
    b_ih = np.asarray(b_ih, np.float64)
    b_hh = np.asarray(b_hh, np.float64)

    W_b = W_ih[:, HIDDEN:] @ W_dec
    b_comb = W_ih[:, HIDDEN:] @ b_dec + b_ih

    # encoder difference: row j = W_enc[2j] - W_enc[2j+1]
    W_diff = W_enc[0::2] - W_enc[1::2]                       # [64, 256]
    b_diff = (b_enc[0::2] - b_enc[1::2]).astype(np.float32)  # [64]
    wdT = np.ascontiguousarray(W_diff.T).astype(bf16)        # [256, 64]
    waT = np.ascontiguousarray(W_ih[:, :HIDDEN].T).astype(bf16)   # [128, 384]
    wbT = np.ascontiguousarray(W_b.T[PERM, :]).astype(bf16)  # [128, 384] perm rows
    whhT = np.ascontiguousarray(W_hh.T).astype(bf16)         # [128, 384]
    brz = (b_comb[:256] + b_hh[:256]).astype(np.float32)
    bn = b_comb[256:384].astype(np.float32)
    bhn = b_hh[256:384].astype(np.float32)
    return {
        "wdT": wdT, "waT": waT, "wbT": wbT, "whhT": whhT,
        "bias_enc": np.ascontiguousarray(np.tile(b_diff, (128, 1))),
        "bias_rz": np.ascontiguousarray(np.tile(brz, (128, 1))),
        "bias_n": np.ascontiguousarray(np.tile(bn, (128, 1))),
        "bias_hn": np.ascontiguousarray(np.tile(bhn, (128, 1))),
    }


def kernel(feat, h, src, dst, u, W_enc, b_enc, W_dec, b_dec, W_ih, W_hh,
           b_ih, b_hh):
    B, in_maps = _prep_host(feat, h, src, dst, u)
    wmap = _prep_weights(W_enc, b_enc, W_dec, b_dec, W_ih, W_hh, b_ih, b_hh)
    for m in in_maps:
        m.update(wmap)

    phases = os.environ.get("KERNEL_PHASES", "zmg")
    zero_bias = not (np.any(np.asarray(b_enc)) or np.any(np.asarray(b_dec))
                     or np.any(np.asarray(b_ih)) or np.any(np.asarray(b_hh)))
    key = (B, phases, zero_bias)
    if key not in _cache:
        _cache[key] = build_program(B, phases, zero_bias)
    nc = _cache[key]

    res = run_bass_kernel_spmd(nc, in_maps, core_ids=list(range(NDEV)))
    h_new = np.concatenate([res.results[d]["h_new"] for d in range(NDEV)], axis=0)
    return (h_new, h_new)
